# revision 1
# baseline (speedup 1.0000x reference)
"""GCN node classifier (2x spmm + classifier + log_softmax) on 8 trn2 cores.

Strategy: destination-node 1D sharding. Each core owns 12,500 dst nodes and
the edges pointing at them. Layer tables (x@W1+b1, relu(h1)@W2+b2) are
node-major bf16 rows in DRAM; per-edge source rows are fetched with GPSIMD
dma_gather (int16 indices, so the table is addressed in 4 quarter views).
The segment-sum is a tensor-engine matmul against per-chunk scatter matrices
V[e, dst_lane] = edge_val[e] built on DVE with (iota == ldst) * val.
Between layers the per-shard T2 table is AllGather'ed into a Shared DRAM
tensor. All accumulation is f32 (PSUM); only table values are bf16.
"""

import numpy as np
import ml_dtypes

from contextlib import ExitStack


# ---------------------------------------------------------------- config ---
class Cfg:
    M = 8                 # cores
    N_NODES = 100000
    N_EDGES = 1600000
    IN_DIM = 128
    HID = 64
    NCLS = 40
    SHARD = 12500         # real dst nodes per core
    NT = 98               # dst tiles per core (128 each)
    KSEG = 5              # chunks (of 128 edges) per (tile, quarter) segment
    SLABC = 49            # chunks per gather slab
    X_BF16 = True         # phase-A (x@W1) in bf16
    SINGLE_PACKET = False  # multi-packet gathers (single-packet hangs >~1K idxs)
    NQUEUES = 4           # spread gathers over all 4 SWDGE queues

    @property
    def PADSHARD(self):
        return self.NT * 128

    @property
    def NPAD(self):
        return self.PADSHARD * self.M

    @property
    def QROWS(self):
        return self.NPAD // 4

    @property
    def SEG(self):
        return self.KSEG * 128

    @property
    def CQ(self):
        return self.NT * self.KSEG          # chunks per quarter

    @property
    def NSLAB(self):
        assert self.CQ % self.SLABC == 0
        return self.CQ // self.SLABC        # gather slabs per quarter

    @property
    def CHUNKS(self):
        return 4 * self.CQ

    @property
    def ASLAB(self):
        # phase-A node slab: 2048 nodes (16 x 128)
        assert self.NPAD % 2048 == 0
        return self.NPAD // 2048


CFG = Cfg()


# ------------------------------------------------------------- host plan ---
def _plan(cfg, edge_row, edge_col, edge_val):
    """Bucket/sort/pad edges per core. Returns per-core arrays:
    idx16 [128, 4*CQ*128/16] int16, ldstT [128, CHUNKS] f32, valT [128, CHUNKS] f32.
    """
    M, SHARD, PADSHARD = cfg.M, cfg.SHARD, cfg.PADSHARD
    NT, KSEG, SEG, CQ, QROWS = cfg.NT, cfg.KSEG, cfg.SEG, cfg.CQ, cfg.QROWS

    # padded (table) node id and quarter decomposition of sources
    psrc = (edge_col // SHARD) * PADSHARD + (edge_col % SHARD)
    q_of = psrc // QROWS
    i_of = psrc % QROWS
    core_of = edge_row // SHARD
    dloc = edge_row % SHARD
    t_of = dloc // 128
    l_of = dloc % 128

    L = 4 * CQ * 128
    idx_all, ldst_all, val_all = [], [], []
    for c in range(M):
        sel = core_of == c
        # order: (quarter, tile) segment id
        segid = q_of[sel] * NT + t_of[sel]
        order = np.argsort(segid, kind="stable")
        sid = segid[order]
        idx_s = i_of[sel][order]
        l_s = l_of[sel][order]
        v_s = edge_val[sel][order]

        counts = np.bincount(sid, minlength=4 * NT)
        if counts.max() > SEG:
            raise ValueError(f"segment overflow: {counts.max()} > {SEG}")
        # place into padded stream: segment s at offset s*SEG
        starts = np.arange(4 * NT) * SEG
        pos = starts[sid] + (np.arange(sid.size) - np.concatenate(([0], np.cumsum(counts)))[sid])

        idx = np.zeros(L, dtype=np.int16)
        ldst = np.zeros(L, dtype=np.float32)
        val = np.zeros(L, dtype=np.float32)
        idx[pos] = idx_s.astype(np.int16)
        ldst[pos] = l_s.astype(np.float32)
        val[pos] = v_s.astype(np.float32)

        # wrap indices: idx i -> [i%16, i//16], replicated on all 8 q7 cores
        idxw = np.tile(idx.reshape(-1, 16).T, (8, 1)).copy()          # [128, L/16]
        ldstT = np.ascontiguousarray(ldst.reshape(-1, 128).T)         # [128, CHUNKS]
        valT = np.ascontiguousarray(val.reshape(-1, 128).T)
        idx_all.append(idxw)
        ldst_all.append(ldstT)
        val_all.append(valT)
    return idx_all, ldst_all, val_all


def _pack_x(cfg, x):
    """x [N, IN] -> padded transposed [IN, NPAD] (f32 or bf16)."""
    xp = np.zeros((cfg.NPAD, cfg.IN_DIM), dtype=np.float32)
    xp.reshape(cfg.M, cfg.PADSHARD, cfg.IN_DIM)[:, : cfg.SHARD] = x.reshape(
        cfg.M, cfg.SHARD, cfg.IN_DIM
    )
    xT = np.ascontiguousarray(xp.T)
    if cfg.X_BF16:
        xT = xT.astype(ml_dtypes.bfloat16)
    return xT


# --------------------------------------------------------- device program ---
def _build(cfg, timing=False):
    from concourse import bacc, tile
    import concourse.mybir as mybir

    f32 = mybir.dt.float32
    bf16 = mybir.dt.bfloat16
    i16 = mybir.dt.int16
    AOP = mybir.AluOpType
    ACT = mybir.ActivationFunctionType

    xdt = bf16 if cfg.X_BF16 else f32

    nc = bacc.Bacc("TRN2", target_bir_lowering=False, debug=False,
                   num_devices=1 if timing else cfg.M,
                   dynamic_dma_scratch_size=getattr(cfg, "DMA_SCRATCH", 16384),
                   num_swdge_queues=getattr(cfg, "NQUEUES", 1))

    NPAD, QROWS, NT, KSEG, CQ, SLABC, NSLAB = (
        cfg.NPAD, cfg.QROWS, cfg.NT, cfg.KSEG, cfg.CQ, cfg.SLABC, cfg.NSLAB)
    CHUNKS, HID, NCLS, IN_DIM = cfg.CHUNKS, cfg.HID, cfg.NCLS, cfg.IN_DIM
    LQ16 = CQ * 128 // 16              # idx columns per quarter
    SLAB16 = SLABC * 128 // 16         # idx columns per slab
    NA = cfg.ASLAB                     # phase-A slabs (2048 nodes each)

    # -------- I/O
    XT = nc.dram_tensor("xt", [IN_DIM, NPAD], xdt, kind="ExternalInput")
    IDX = nc.dram_tensor("idx", [128, 4 * LQ16], i16, kind="ExternalInput")
    LDST = nc.dram_tensor("ldst", [128, CHUNKS], f32, kind="ExternalInput")
    VAL = nc.dram_tensor("val", [128, CHUNKS], f32, kind="ExternalInput")
    W1 = nc.dram_tensor("w1", [IN_DIM, HID], xdt, kind="ExternalInput")
    W2 = nc.dram_tensor("w2", [HID, HID], f32, kind="ExternalInput")
    WC = nc.dram_tensor("wc", [HID, NCLS], f32, kind="ExternalInput")
    B1 = nc.dram_tensor("b1", [128, HID], f32, kind="ExternalInput")   # replicated
    B2 = nc.dram_tensor("b2", [128, HID], f32, kind="ExternalInput")
    BC = nc.dram_tensor("bc", [128, NCLS], f32, kind="ExternalInput")
    IOTA = nc.dram_tensor("iota", [128, 128], bf16, kind="ExternalInput")
    IDENT = nc.dram_tensor("ident", [128, 128], f32, kind="ExternalInput")
    OUT = nc.dram_tensor("out", [cfg.PADSHARD, NCLS], f32, kind="ExternalOutput")

    # -------- internal DRAM
    T1 = nc.dram_tensor("t1tab", [NPAD, 128], bf16)                 # cols 64: junk
    T2S = nc.dram_tensor("t2shard", [cfg.PADSHARD, 128], bf16)
    T2F = nc.dram_tensor("t2full", [NPAD, 128], bf16, addr_space="Shared")

    with tile.TileContext(nc) as tc, ExitStack() as top:
        cpool = top.enter_context(tc.tile_pool(name="consts", bufs=1))
        w1s = cpool.tile([IN_DIM, HID], xdt)
        nc.sync.dma_start(out=w1s, in_=W1[:, :])
        w2s = cpool.tile([HID, HID], f32)
        nc.sync.dma_start(out=w2s, in_=W2[:, :])
        wcs = cpool.tile([HID, NCLS], f32)
        nc.sync.dma_start(out=wcs, in_=WC[:, :])
        b1s = cpool.tile([128, HID], f32)
        nc.sync.dma_start(out=b1s, in_=B1[:, :])
        b2s = cpool.tile([128, HID], f32)
        nc.sync.dma_start(out=b2s, in_=B2[:, :])
        bcs = cpool.tile([128, NCLS], f32)
        nc.sync.dma_start(out=bcs, in_=BC[:, :])
        b18s = cpool.tile([128, 8, HID], f32)
        for r in range(8):
            nc.sync.dma_start(out=b18s[:, r, :], in_=B1[:, :])
        iot = cpool.tile([128, 128], bf16)
        nc.sync.dma_start(out=iot, in_=IOTA[:, :])
        idn = cpool.tile([128, 128], f32)
        nc.sync.dma_start(out=idn, in_=IDENT[:, :])

        edg = top.enter_context(tc.tile_pool(name="edg", bufs=1))
        ldsts = edg.tile([128, CHUNKS], f32)
        nc.sync.dma_start(out=ldsts, in_=LDST[:, :])
        vals = edg.tile([128, CHUNKS], f32)
        nc.sync.dma_start(out=vals, in_=VAL[:, :])

        accp = top.enter_context(tc.tile_pool(name="acc", bufs=1))

        # ================= phase A: T1 = x @ W1 + b1 (node-major bf16 rows)
        with tc.tile_pool(name="xa", bufs=3) as xa, \
             tc.tile_pool(name="sta", bufs=3) as sta, \
             tc.tile_pool(name="psa", bufs=4, space="PSUM") as psa:
            for s in range(NA):
                xs = xa.tile([128, 2048], xdt)
                nc.sync.dma_start(out=xs, in_=XT[:, s * 2048:(s + 1) * 2048])
                st = sta.tile([128, 16, HID], bf16)
                for h in range(2):
                    pb = psa.tile([128, 8, HID], f32)
                    for k8 in range(8):
                        k = h * 8 + k8
                        nc.tensor.matmul(pb[:, k8, :],
                                         lhsT=xs[:, k * 128:(k + 1) * 128],
                                         rhs=w1s, start=True, stop=True)
                    nc.vector.tensor_tensor(st[:, h * 8:(h + 1) * 8, :], pb,
                                            b18s, AOP.add)
                dst = T1[s * 2048:(s + 1) * 2048, 0:HID].rearrange(
                    "(k p) f -> p k f", p=128)
                nc.sync.dma_start(out=dst, in_=st)

        # ============ spmm layer runner: per-tile single psum group across
        # all 4 quarters (slabs for all quarters retire in lockstep), with a
        # fused per-tile epilogue.
        def spmm_layer(tab, epilogue, pools):
            idxp, msg, vp, psb = pools
            its = []
            slabs = [[None] * NSLAB for _ in range(4)]
            for q in range(4):
                it = idxp.tile([128, LQ16], i16, tag=f"idx{q}")
                nc.sync.dma_start(out=it, in_=IDX[:, q * LQ16:(q + 1) * LQ16])
                its.append(it)

            def ensure_slab(q, s):
                if slabs[q][s] is None:
                    mt = msg.tile([128, SLABC, 128], bf16)
                    nc.gpsimd.dma_gather(
                        mt, tab[q * QROWS:(q + 1) * QROWS, :],
                        its[q][:, s * SLAB16:(s + 1) * SLAB16],
                        num_idxs=SLABC * 128, num_idxs_reg=SLABC * 128,
                        elem_size=128, elem_step=128,
                        single_packet=getattr(cfg, "SINGLE_PACKET", True),
                        queue_num=(q * NSLAB + s) % getattr(cfg, "NQUEUES", 1))
                    slabs[q][s] = mt
                return slabs[q][s]

            for t in range(NT):
                ps = psb.tile([128, HID], f32)
                for q in range(4):
                    for k in range(KSEG):
                        j = t * KSEG + k                 # chunk in quarter
                        gj = q * CQ + j                  # global chunk
                        v = vp.tile([128, 128], bf16)
                        nc.vector.tensor_scalar(
                            v, iot, ldsts[:, gj:gj + 1], vals[:, gj:gj + 1],
                            AOP.is_equal, AOP.mult)
                        mt = ensure_slab(q, j // SLABC)
                        nc.tensor.matmul(ps, lhsT=v,
                                         rhs=mt[:, j % SLABC, 0:HID],
                                         start=(q == 0 and k == 0),
                                         stop=(q == 3 and k == KSEG - 1))
                epilogue(t, ps)

        # ================= layer 1 + phase C fused: T2S = relu(h1)@W2+b2
        for _rep in range(getattr(cfg, "REPS", 1)):
            _run_layers(cfg, nc, tc, tile, mybir, timing, accp, locals())
    nc.compile()
    return nc


def _run_layers(cfg, nc, tc, tile, mybir, timing, accp, env):
    f32 = mybir.dt.float32
    bf16 = mybir.dt.bfloat16
    i16 = mybir.dt.int16
    AOP = mybir.AluOpType
    ACT = mybir.ActivationFunctionType
    NPAD, QROWS, NT, KSEG, CQ, SLABC, NSLAB = (
        cfg.NPAD, cfg.QROWS, cfg.NT, cfg.KSEG, cfg.CQ, cfg.SLABC, cfg.NSLAB)
    CHUNKS, HID, NCLS, IN_DIM = cfg.CHUNKS, cfg.HID, cfg.NCLS, cfg.IN_DIM
    LQ16 = CQ * 128 // 16
    SLAB16 = SLABC * 128 // 16
    (T1, T2S, T2F, IDX, OUT, iot, idn, ldsts, vals, w2s, wcs, b2s, bcs) = (
        env["T1"], env["T2S"], env["T2F"], env["IDX"], env["OUT"], env["iot"],
        env["idn"], env["ldsts"], env["vals"], env["w2s"], env["wcs"],
        env["b2s"], env["bcs"])
    spmm_layer = env["spmm_layer"]

    if True:
        with tc.tile_pool(name="idxp", bufs=getattr(cfg, "IDXBUFS", 2)) as idxp, \
             tc.tile_pool(name="msg", bufs=getattr(cfg, "MSGBUFS", 8)) as msg, \
             tc.tile_pool(name="vp", bufs=8) as vp, \
             tc.tile_pool(name="psb", bufs=3, space="PSUM") as psb, \
             tc.tile_pool(name="tc1", bufs=3) as tp1, \
             tc.tile_pool(name="tc2", bufs=3) as tp2, \
             tc.tile_pool(name="tc3", bufs=3) as tp3, \
             tc.tile_pool(name="pst", bufs=2, space="PSUM") as pst, \
             tc.tile_pool(name="psc", bufs=2, space="PSUM") as psc:

            def epi1(t, ps):
                h1r = tp1.tile([128, HID], f32)
                nc.scalar.activation(h1r, ps, ACT.Relu)
                ptr = pst.tile([HID, 128], f32)
                nc.tensor.transpose(ptr, h1r, idn)
                h1t = tp2.tile([HID, 128], f32)
                nc.vector.tensor_copy(out=h1t, in_=ptr)
                ps2 = psc.tile([128, HID], f32)
                nc.tensor.matmul(ps2, lhsT=h1t, rhs=w2s, start=True, stop=True)
                t2t = tp3.tile([128, HID], bf16)
                nc.vector.tensor_tensor(t2t, ps2, b2s, AOP.add)
                nc.sync.dma_start(out=T2S[t * 128:(t + 1) * 128, 0:HID], in_=t2t)

            spmm_layer(T1, epi1, (idxp, msg, vp, psb))
            if not timing:
                nc.gpsimd.collective_compute(
                    "AllGather", mybir.AluOpType.bypass,
                    replica_groups=[list(range(cfg.M))],
                    ins=[T2S[:, :]], outs=[T2F[:, :]])

        # ================= layer 2 + phase E fused: logits + log_softmax
        with tc.tile_pool(name="idxp2", bufs=getattr(cfg, "IDXBUFS", 2)) as idxp2, \
             tc.tile_pool(name="msg2", bufs=getattr(cfg, "MSGBUFS", 8)) as msg2, \
             tc.tile_pool(name="vp2", bufs=8) as vp2, \
             tc.tile_pool(name="psb2", bufs=3, space="PSUM") as psb2, \
             tc.tile_pool(name="te1", bufs=3) as te1, \
             tc.tile_pool(name="te2", bufs=3) as te2, \
             tc.tile_pool(name="pse", bufs=2, space="PSUM") as pse, \
             tc.tile_pool(name="psf", bufs=2, space="PSUM") as psf:
            lgacc = accp.tile([128, NT, NCLS], f32, tag="lgacc")
            negmacc = accp.tile([128, NT], f32, tag="negmacc")
            smacc = accp.tile([128, NT], f32, tag="smacc")

            def epi2(t, ps):
                h2s = te1.tile([128, HID], f32)
                nc.scalar.activation(h2s, ps, ACT.Copy)
                ptr = pse.tile([HID, 128], f32)
                nc.tensor.transpose(ptr, h2s, idn)
                h2t = te2.tile([HID, 128], f32)
                nc.vector.tensor_copy(out=h2t, in_=ptr)
                psl = psf.tile([128, NCLS], f32)
                nc.tensor.matmul(psl, lhsT=h2t, rhs=wcs, start=True, stop=True)
                nc.vector.tensor_tensor(lgacc[:, t, :], psl, bcs, AOP.add)
                nc.vector.tensor_reduce(negmacc[:, t:t + 1], lgacc[:, t, :],
                                        mybir.AxisListType.X, AOP.max,
                                        negate=True)
                et = te1.tile([128, NCLS], f32, tag="et")
                nc.scalar.activation(et, lgacc[:, t, :], ACT.Exp,
                                     bias=negmacc[:, t:t + 1],
                                     accum_out=smacc[:, t:t + 1])

            spmm_layer(T2F, epi2, (idxp2, msg2, vp2, psb2))

            # one Ln over all tiles, then final subtract + store
            lnacc = accp.tile([128, NT], f32, tag="lnacc")
            nc.scalar.activation(lnacc, smacc, ACT.Ln)
            shacc = accp.tile([128, NT], f32, tag="shacc")
            nc.vector.tensor_tensor(shacc, lnacc, negmacc, AOP.subtract)
            for t in range(NT):
                ot = te2.tile([128, NCLS], f32, tag="ot")
                nc.vector.tensor_scalar(ot, lgacc[:, t, :],
                                        shacc[:, t:t + 1], None, AOP.subtract)
                nc.sync.dma_start(out=OUT[t * 128:(t + 1) * 128, :], in_=ot)

    nc.compile()
    return nc


_NC_CACHE = {}


def _get_nc(cfg):
    key = (cfg.KSEG, cfg.X_BF16, getattr(cfg, "REPS", 1), cfg.SLABC)
    if key not in _NC_CACHE:
        _NC_CACHE[key] = _build(cfg)
    return _NC_CACHE[key]


# ------------------------------------------------------------------ main ---
def kernel(x, edge_row, edge_col, edge_val, W1, b1, W2, b2, Wc, bc,
           _run_kwargs=None):
    from concourse.bass_utils import run_bass_kernel_spmd

    cfg = CFG
    x = np.asarray(x, dtype=np.float32)
    edge_row = np.asarray(edge_row, dtype=np.int64)
    edge_col = np.asarray(edge_col, dtype=np.int64)
    edge_val = np.asarray(edge_val, dtype=np.float32)
    W1 = np.asarray(W1, dtype=np.float32)
    W2 = np.asarray(W2, dtype=np.float32)
    Wc = np.asarray(Wc, dtype=np.float32)
    b1 = np.asarray(b1, dtype=np.float32)
    b2 = np.asarray(b2, dtype=np.float32)
    bc = np.asarray(bc, dtype=np.float32)

    try:
        idx_all, ldst_all, val_all = _plan(cfg, edge_row, edge_col, edge_val)
    except ValueError:
        cfg.KSEG += 1
        idx_all, ldst_all, val_all = _plan(cfg, edge_row, edge_col, edge_val)

    xT = _pack_x(cfg, x)
    w1h = W1.astype(ml_dtypes.bfloat16) if cfg.X_BF16 else W1
    iota = np.tile(np.arange(128, dtype=np.float32), (128, 1)).astype(
        ml_dtypes.bfloat16)
    ident = np.eye(128, dtype=np.float32)
    b1r = np.tile(b1, (128, 1)).astype(np.float32)
    b2r = np.tile(b2, (128, 1)).astype(np.float32)
    bcr = np.tile(bc, (128, 1)).astype(np.float32)

    nc = _get_nc(cfg)
    in_maps = []
    for c in range(cfg.M):
        in_maps.append({
            "xt": xT, "idx": idx_all[c], "ldst": ldst_all[c],
            "val": val_all[c], "w1": w1h, "w2": W2, "wc": Wc,
            "b1": b1r, "b2": b2r, "bc": bcr, "iota": iota, "ident": ident,
        })
    kw = dict(_run_kwargs or {})
    res = run_bass_kernel_spmd(nc, in_maps, core_ids=list(range(cfg.M)), **kw)
    out = np.concatenate(
        [res.results[c]["out"][: cfg.SHARD] for c in range(cfg.M)], axis=0)
    kernel.last_results = res
    return out.astype(np.float32)



# revision 15
# speedup vs baseline: 1.1420x; 1.1420x over previous
"""GCN node classifier (2x spmm + classifier + log_softmax) on 8 trn2 cores.

Strategy: destination-node 1D sharding. Each core owns 12,500 dst nodes and
the edges pointing at them. Layer tables (x@W1+b1, relu(h1)@W2+b2) are
node-major bf16 rows in DRAM; per-edge source rows are fetched with GPSIMD
dma_gather (int16 indices, so the table is addressed in 4 quarter views).
The segment-sum is a tensor-engine matmul against per-chunk scatter matrices
V[e, dst_lane] = edge_val[e] built on DVE with (iota == ldst) * val.
Between layers the per-shard T2 table is AllGather'ed into a Shared DRAM
tensor. All accumulation is f32 (PSUM); only table values are bf16.
"""

import numpy as np
import ml_dtypes

from contextlib import ExitStack


# ---------------------------------------------------------------- config ---
class Cfg:
    M = 8                 # cores
    N_NODES = 100000
    N_EDGES = 1600000
    IN_DIM = 128
    HID = 64
    NCLS = 40
    SHARD = 12500         # real dst nodes per core
    NT = 98               # dst tiles per core (128 each)
    KSEG = 5              # chunks (of 128 edges) per (tile, quarter) segment
    SLABC = 49            # chunks per gather slab
    X_BF16 = True         # phase-A (x@W1) in bf16
    SINGLE_PACKET = False  # multi-packet gathers (single-packet hangs >~1K idxs)
    NQUEUES = 4           # spread gathers over all 4 SWDGE queues

    @property
    def PADSHARD(self):
        return self.NT * 128

    @property
    def NPAD(self):
        return self.PADSHARD * self.M

    @property
    def QROWS(self):
        return self.NPAD // 4

    @property
    def SEG(self):
        return self.KSEG * 128

    @property
    def CQ(self):
        return self.NT * self.KSEG          # chunks per quarter

    @property
    def NSLAB(self):
        assert self.CQ % self.SLABC == 0
        return self.CQ // self.SLABC        # gather slabs per quarter

    @property
    def CHUNKS(self):
        return 4 * self.CQ

    @property
    def ASLAB(self):
        # phase-A node slab: 1792 own-shard nodes (14 x 128)
        assert self.PADSHARD % 1792 == 0
        return self.PADSHARD // 1792


CFG = Cfg()


# ------------------------------------------------------------- host plan ---
def _plan(cfg, edge_row, edge_col, edge_val):
    """Bucket/sort/pad edges per core. Returns per-core arrays:
    idx16 [128, 4*CQ*128/16] int16, ldstT [128, CHUNKS] f32, valT [128, CHUNKS] f32.
    """
    M, SHARD, PADSHARD = cfg.M, cfg.SHARD, cfg.PADSHARD
    NT, KSEG, SEG, CQ, QROWS = cfg.NT, cfg.KSEG, cfg.SEG, cfg.CQ, cfg.QROWS

    # padded (table) node id and quarter decomposition of sources
    psrc = (edge_col // SHARD) * PADSHARD + (edge_col % SHARD)
    q_of = psrc // QROWS
    i_of = psrc % QROWS
    core_of = edge_row // SHARD
    dloc = edge_row % SHARD
    t_of = dloc // 128
    l_of = dloc % 128

    L = 4 * CQ * 128
    idx_all, ldst_all, val_all = [], [], []
    for c in range(M):
        sel = core_of == c
        # order: (quarter, tile) segment id
        segid = q_of[sel] * NT + t_of[sel]
        order = np.argsort(segid, kind="stable")
        sid = segid[order]
        idx_s = i_of[sel][order]
        l_s = l_of[sel][order]
        v_s = edge_val[sel][order]

        counts = np.bincount(sid, minlength=4 * NT)
        if counts.max() > SEG:
            raise ValueError(f"segment overflow: {counts.max()} > {SEG}")
        # place into padded stream: segment s at offset s*SEG
        starts = np.arange(4 * NT) * SEG
        pos = starts[sid] + (np.arange(sid.size) - np.concatenate(([0], np.cumsum(counts)))[sid])

        idx = np.zeros(L, dtype=np.int16)
        ldst = np.zeros(L, dtype=np.float32)
        val = np.zeros(L, dtype=np.float32)
        idx[pos] = idx_s.astype(np.int16)
        ldst[pos] = l_s.astype(np.float32)
        val[pos] = v_s.astype(np.float32)

        # wrap indices: idx i -> [i%16, i//16], replicated on all 8 q7 cores
        idxw = np.tile(idx.reshape(-1, 16).T, (8, 1)).copy()          # [128, L/16]
        ldstT = np.ascontiguousarray(ldst.reshape(-1, 128).T)        # [128, CHUNKS]
        valT = np.ascontiguousarray(val.reshape(-1, 128).T)
        idx_all.append(idxw)
        ldst_all.append(ldstT)
        val_all.append(valT)
    return idx_all, ldst_all, val_all


def _pack_x(cfg, x):
    """x [N, IN] -> per-core padded transposed shards [IN, PADSHARD]."""
    shards = []
    for c in range(cfg.M):
        xp = np.zeros((cfg.PADSHARD, cfg.IN_DIM), dtype=np.float32)
        xp[: cfg.SHARD] = x[c * cfg.SHARD:(c + 1) * cfg.SHARD]
        xT = np.ascontiguousarray(xp.T)
        if cfg.X_BF16:
            xT = xT.astype(ml_dtypes.bfloat16)
        shards.append(xT)
    return shards


# --------------------------------------------------------- device program ---
def _build(cfg, timing=False):
    from concourse import bacc, tile
    import concourse.mybir as mybir

    f32 = mybir.dt.float32
    bf16 = mybir.dt.bfloat16
    i16 = mybir.dt.int16
    AOP = mybir.AluOpType
    ACT = mybir.ActivationFunctionType

    xdt = bf16 if cfg.X_BF16 else f32

    nc = bacc.Bacc("TRN2", target_bir_lowering=False, debug=False,
                   num_devices=1 if timing else cfg.M,
                   dynamic_dma_scratch_size=getattr(cfg, "DMA_SCRATCH", 16384),
                   num_swdge_queues=getattr(cfg, "NQUEUES", 1))

    NPAD, QROWS, NT, KSEG, CQ, SLABC, NSLAB = (
        cfg.NPAD, cfg.QROWS, cfg.NT, cfg.KSEG, cfg.CQ, cfg.SLABC, cfg.NSLAB)
    CHUNKS, HID, NCLS, IN_DIM = cfg.CHUNKS, cfg.HID, cfg.NCLS, cfg.IN_DIM
    LQ16 = CQ * 128 // 16              # idx columns per quarter
    SLAB16 = SLABC * 128 // 16         # idx columns per slab
    NA = cfg.ASLAB                     # phase-A slabs (2048 nodes each)

    # -------- I/O
    XT = nc.dram_tensor("xt", [IN_DIM, cfg.PADSHARD], xdt, kind="ExternalInput")
    IDX = nc.dram_tensor("idx", [128, 4 * LQ16], i16, kind="ExternalInput")
    LDST = nc.dram_tensor("ldst", [128, CHUNKS], f32, kind="ExternalInput")
    VAL = nc.dram_tensor("val", [128, CHUNKS], f32, kind="ExternalInput")
    W1 = nc.dram_tensor("w1", [IN_DIM, HID], xdt, kind="ExternalInput")
    W2 = nc.dram_tensor("w2", [HID, HID], f32, kind="ExternalInput")
    WC = nc.dram_tensor("wc", [HID, NCLS], f32, kind="ExternalInput")
    B1 = nc.dram_tensor("b1", [128, HID], f32, kind="ExternalInput")   # replicated
    B2 = nc.dram_tensor("b2", [128, HID], f32, kind="ExternalInput")
    BC = nc.dram_tensor("bc", [128, NCLS], f32, kind="ExternalInput")
    IOTA = nc.dram_tensor("iota", [128, 128], bf16, kind="ExternalInput")
    IDENT = nc.dram_tensor("ident", [128, 128], f32, kind="ExternalInput")
    OUT = nc.dram_tensor("out", [cfg.PADSHARD, NCLS], f32, kind="ExternalOutput")

    # -------- internal DRAM
    T1S = nc.dram_tensor("t1shard", [cfg.PADSHARD, 128], bf16)      # cols 64: junk
    T1F = nc.dram_tensor("t1full", [NPAD, 128], bf16, addr_space="Shared")
    T2S = nc.dram_tensor("t2shard", [cfg.PADSHARD, 128], bf16)
    T2F = nc.dram_tensor("t2full", [NPAD, 128], bf16, addr_space="Shared")

    with tile.TileContext(nc) as tc, ExitStack() as top:
        cpool = top.enter_context(tc.tile_pool(name="consts", bufs=1))
        w1s = cpool.tile([IN_DIM, HID], xdt)
        nc.sync.dma_start(out=w1s, in_=W1[:, :])
        w2s = cpool.tile([HID, HID], f32)
        nc.sync.dma_start(out=w2s, in_=W2[:, :])
        wcs = cpool.tile([HID, NCLS], f32)
        nc.sync.dma_start(out=wcs, in_=WC[:, :])
        b1s = cpool.tile([128, HID], f32)
        nc.sync.dma_start(out=b1s, in_=B1[:, :])
        b2s = cpool.tile([128, HID], f32)
        nc.sync.dma_start(out=b2s, in_=B2[:, :])
        bcs = cpool.tile([128, NCLS], f32)
        nc.sync.dma_start(out=bcs, in_=BC[:, :])
        b17s = cpool.tile([128, 7, HID], f32)
        for r in range(7):
            nc.sync.dma_start(out=b17s[:, r, :], in_=B1[:, :])
        iot = cpool.tile([128, 128], bf16)
        nc.sync.dma_start(out=iot, in_=IOTA[:, :])
        idn = cpool.tile([128, 128], f32)
        nc.sync.dma_start(out=idn, in_=IDENT[:, :])

        edg = top.enter_context(tc.tile_pool(name="edg", bufs=1))
        ldsts = edg.tile([128, CHUNKS], f32)
        nc.sync.dma_start(out=ldsts, in_=LDST[:, :])
        vals = edg.tile([128, CHUNKS], f32)
        nc.sync.dma_start(out=vals, in_=VAL[:, :])
        # persistent per-quarter edge-gather indices (used by both layers)
        its = []
        for q in range(4):
            it = edg.tile([128, LQ16], i16, tag=f"idx{q}")
            nc.sync.dma_start(out=it, in_=IDX[:, q * LQ16:(q + 1) * LQ16])
            its.append(it)

        accp = top.enter_context(tc.tile_pool(name="acc", bufs=1))

        # ====== phase A: T1S = x_shard @ W1 + b1 (node-major bf16 rows),
        # then AllGather into the full table T1F.
        with tc.tile_pool(name="xa", bufs=3) as xa, \
             tc.tile_pool(name="sta", bufs=3) as sta, \
             tc.tile_pool(name="psa", bufs=4, space="PSUM") as psa:
            for s in range(NA):
                xs = xa.tile([128, 1792], xdt)
                nc.sync.dma_start(out=xs, in_=XT[:, s * 1792:(s + 1) * 1792])
                st = sta.tile([128, 14, HID], bf16)
                for h in range(2):
                    pb = psa.tile([128, 7, HID], f32)
                    for k7 in range(7):
                        k = h * 7 + k7
                        nc.tensor.matmul(pb[:, k7, :],
                                         lhsT=xs[:, k * 128:(k + 1) * 128],
                                         rhs=w1s, start=True, stop=True)
                    nc.vector.tensor_tensor(st[:, h * 7:(h + 1) * 7, :], pb,
                                            b17s, AOP.add)
                dst = T1S[s * 1792:(s + 1) * 1792, 0:HID].rearrange(
                    "(k p) f -> p k f", p=128)
                nc.sync.dma_start(out=dst, in_=st)
        if not timing:
            nc.gpsimd.collective_compute(
                "AllGather", mybir.AluOpType.bypass,
                replica_groups=[list(range(cfg.M))],
                ins=[T1S[:, :]], outs=[T1F[:, :]])

        # ============ spmm layer runner: per-tile single psum group across
        # all 4 quarters (slabs for all quarters retire in lockstep), with a
        # fused per-tile epilogue.
        def spmm_layer(tab, epilogue, pools):
            msg, vp, psb = pools
            slabs = [[None] * NSLAB for _ in range(4)]

            def ensure_slab(q, s):
                if slabs[q][s] is None:
                    mt = msg.tile([128, SLABC, 128], bf16)
                    nc.gpsimd.dma_gather(
                        mt, tab[q * QROWS:(q + 1) * QROWS, :],
                        its[q][:, s * SLAB16:(s + 1) * SLAB16],
                        num_idxs=SLABC * 128, num_idxs_reg=SLABC * 128,
                        elem_size=128, elem_step=128,
                        single_packet=getattr(cfg, "SINGLE_PACKET", True),
                        queue_num=(q * NSLAB + s) % getattr(cfg, "NQUEUES", 1))
                    slabs[q][s] = mt
                return slabs[q][s]

            for t in range(NT):
                ps = psb.tile([128, HID], f32)
                for q in range(4):
                    for k in range(KSEG):
                        j = t * KSEG + k                 # chunk in quarter
                        gj = q * CQ + j                  # global chunk
                        v = vp.tile([128, 128], bf16)
                        nc.vector.tensor_scalar(
                            v, iot, ldsts[:, gj:gj + 1], vals[:, gj:gj + 1],
                            AOP.is_equal, AOP.mult)
                        mt = ensure_slab(q, j // SLABC)
                        nc.tensor.matmul(ps, lhsT=v,
                                         rhs=mt[:, j % SLABC, 0:HID],
                                         start=(q == 0 and k == 0),
                                         stop=(q == 3 and k == KSEG - 1))
                epilogue(t, ps)

        # ================= layer 1 + phase C fused: T2S = relu(h1)@W2+b2
        for _rep in range(getattr(cfg, "REPS", 1)):
            _run_layers(cfg, nc, tc, tile, mybir, timing, accp, locals())
    nc.compile()
    return nc


def _run_layers(cfg, nc, tc, tile, mybir, timing, accp, env):
    f32 = mybir.dt.float32
    bf16 = mybir.dt.bfloat16
    i16 = mybir.dt.int16
    AOP = mybir.AluOpType
    ACT = mybir.ActivationFunctionType
    NPAD, QROWS, NT, KSEG, CQ, SLABC, NSLAB = (
        cfg.NPAD, cfg.QROWS, cfg.NT, cfg.KSEG, cfg.CQ, cfg.SLABC, cfg.NSLAB)
    CHUNKS, HID, NCLS, IN_DIM = cfg.CHUNKS, cfg.HID, cfg.NCLS, cfg.IN_DIM
    LQ16 = CQ * 128 // 16
    SLAB16 = SLABC * 128 // 16
    (T1F, T2S, T2F, IDX, OUT, iot, idn, ldsts, vals, w2s, wcs, b2s, bcs) = (
        env["T1F"], env["T2S"], env["T2F"], env["IDX"], env["OUT"], env["iot"],
        env["idn"], env["ldsts"], env["vals"], env["w2s"], env["wcs"],
        env["b2s"], env["bcs"])
    spmm_layer = env["spmm_layer"]

    if True:
        with tc.tile_pool(name="msg", bufs=getattr(cfg, "MSGBUFS", 8)) as msg, \
             tc.tile_pool(name="vp", bufs=8) as vp, \
             tc.tile_pool(name="psb", bufs=3, space="PSUM") as psb, \
             tc.tile_pool(name="tc1", bufs=3) as tp1, \
             tc.tile_pool(name="tc2", bufs=3) as tp2, \
             tc.tile_pool(name="tc3", bufs=3) as tp3, \
             tc.tile_pool(name="pst", bufs=2, space="PSUM") as pst, \
             tc.tile_pool(name="psc", bufs=2, space="PSUM") as psc:

            def epi1(t, ps):
                h1r = tp1.tile([128, HID], f32)
                nc.scalar.activation(h1r, ps, ACT.Relu)
                ptr = pst.tile([HID, 128], f32)
                nc.tensor.transpose(ptr, h1r, idn)
                h1t = tp2.tile([HID, 128], f32)
                nc.vector.tensor_copy(out=h1t, in_=ptr)
                ps2 = psc.tile([128, HID], f32)
                nc.tensor.matmul(ps2, lhsT=h1t, rhs=w2s, start=True, stop=True)
                t2t = tp3.tile([128, HID], bf16)
                nc.vector.tensor_tensor(t2t, ps2, b2s, AOP.add)
                nc.sync.dma_start(out=T2S[t * 128:(t + 1) * 128, 0:HID], in_=t2t)

            spmm_layer(T1F, epi1, (msg, vp, psb))
            if not timing:
                nc.gpsimd.collective_compute(
                    "AllGather", mybir.AluOpType.bypass,
                    replica_groups=[list(range(cfg.M))],
                    ins=[T2S[:, :]], outs=[T2F[:, :]])

        # ================= layer 2 + phase E fused: logits + log_softmax
        with tc.tile_pool(name="msg2", bufs=getattr(cfg, "MSGBUFS", 8)) as msg2, \
             tc.tile_pool(name="vp2", bufs=8) as vp2, \
             tc.tile_pool(name="psb2", bufs=3, space="PSUM") as psb2, \
             tc.tile_pool(name="te1", bufs=3) as te1, \
             tc.tile_pool(name="te2", bufs=3) as te2, \
             tc.tile_pool(name="pse", bufs=2, space="PSUM") as pse, \
             tc.tile_pool(name="psf", bufs=2, space="PSUM") as psf:
            lgacc = accp.tile([128, NT, NCLS], f32, tag="lgacc")
            negmacc = accp.tile([128, NT], f32, tag="negmacc")
            smacc = accp.tile([128, NT], f32, tag="smacc")

            def epi2(t, ps):
                h2s = te1.tile([128, HID], f32)
                nc.scalar.activation(h2s, ps, ACT.Copy)
                ptr = pse.tile([HID, 128], f32)
                nc.tensor.transpose(ptr, h2s, idn)
                h2t = te2.tile([HID, 128], f32)
                nc.vector.tensor_copy(out=h2t, in_=ptr)
                psl = psf.tile([128, NCLS], f32)
                nc.tensor.matmul(psl, lhsT=h2t, rhs=wcs, start=True, stop=True)
                nc.vector.tensor_tensor(lgacc[:, t, :], psl, bcs, AOP.add)
                nc.vector.tensor_reduce(negmacc[:, t:t + 1], lgacc[:, t, :],
                                        mybir.AxisListType.X, AOP.max,
                                        negate=True)
                et = te1.tile([128, NCLS], f32, tag="et")
                nc.scalar.activation(et, lgacc[:, t, :], ACT.Exp,
                                     bias=negmacc[:, t:t + 1],
                                     accum_out=smacc[:, t:t + 1])

            spmm_layer(T2F, epi2, (msg2, vp2, psb2))

            # one Ln over all tiles, then final subtract + store
            lnacc = accp.tile([128, NT], f32, tag="lnacc")
            nc.scalar.activation(lnacc, smacc, ACT.Ln)
            shacc = accp.tile([128, NT], f32, tag="shacc")
            nc.vector.tensor_tensor(shacc, lnacc, negmacc, AOP.subtract)
            for t in range(NT):
                ot = te2.tile([128, NCLS], f32, tag="ot")
                nc.vector.tensor_scalar(ot, lgacc[:, t, :],
                                        shacc[:, t:t + 1], None, AOP.subtract)
                nc.sync.dma_start(out=OUT[t * 128:(t + 1) * 128, :], in_=ot)

    nc.compile()
    return nc


_NC_CACHE = {}


def _get_nc(cfg):
    key = (cfg.KSEG, cfg.X_BF16, getattr(cfg, "REPS", 1), cfg.SLABC)
    if key not in _NC_CACHE:
        _NC_CACHE[key] = _build(cfg)
    return _NC_CACHE[key]


# ------------------------------------------------------------------ main ---
def kernel(x, edge_row, edge_col, edge_val, W1, b1, W2, b2, Wc, bc,
           _run_kwargs=None):
    from concourse.bass_utils import run_bass_kernel_spmd

    cfg = CFG
    x = np.asarray(x, dtype=np.float32)
    edge_row = np.asarray(edge_row, dtype=np.int64)
    edge_col = np.asarray(edge_col, dtype=np.int64)
    edge_val = np.asarray(edge_val, dtype=np.float32)
    W1 = np.asarray(W1, dtype=np.float32)
    W2 = np.asarray(W2, dtype=np.float32)
    Wc = np.asarray(Wc, dtype=np.float32)
    b1 = np.asarray(b1, dtype=np.float32)
    b2 = np.asarray(b2, dtype=np.float32)
    bc = np.asarray(bc, dtype=np.float32)

    try:
        idx_all, ldst_all, val_all = _plan(cfg, edge_row, edge_col, edge_val)
    except ValueError:
        cfg.KSEG += 1
        idx_all, ldst_all, val_all = _plan(cfg, edge_row, edge_col, edge_val)

    xT = _pack_x(cfg, x)
    w1h = W1.astype(ml_dtypes.bfloat16) if cfg.X_BF16 else W1
    iota = np.tile(np.arange(128, dtype=np.float32), (128, 1)).astype(
        ml_dtypes.bfloat16)
    ident = np.eye(128, dtype=np.float32)
    b1r = np.tile(b1, (128, 1)).astype(np.float32)
    b2r = np.tile(b2, (128, 1)).astype(np.float32)
    bcr = np.tile(bc, (128, 1)).astype(np.float32)

    nc = _get_nc(cfg)
    in_maps = []
    for c in range(cfg.M):
        in_maps.append({
            "xt": xT[c], "idx": idx_all[c], "ldst": ldst_all[c],
            "val": val_all[c], "w1": w1h, "w2": W2, "wc": Wc,
            "b1": b1r, "b2": b2r, "bc": bcr, "iota": iota, "ident": ident,
        })
    kw = dict(_run_kwargs or {})
    res = run_bass_kernel_spmd(nc, in_maps, core_ids=list(range(cfg.M)), **kw)
    out = np.concatenate(
        [res.results[c]["out"][: cfg.SHARD] for c in range(cfg.M)], axis=0)
    kernel.last_results = res
    return out.astype(np.float32)



# revision 23
# speedup vs baseline: 1.4602x; 1.2787x over previous
"""GCN node classifier (2x spmm + classifier + log_softmax) on 8 trn2 cores.

Strategy: destination-node 1D sharding. Each core owns 12,500 dst nodes and
the edges pointing at them. Layer tables (x@W1+b1, relu(h1)@W2+b2) are
node-major bf16 rows in DRAM; per-edge source rows are fetched with GPSIMD
dma_gather (int16 indices, so the table is addressed in 4 quarter views).
The segment-sum is a tensor-engine matmul against per-chunk scatter matrices
V[e, dst_lane] = edge_val[e] built on DVE with (iota == ldst) * val.
Between layers the per-shard T2 table is AllGather'ed into a Shared DRAM
tensor. All accumulation is f32 (PSUM); only table values are bf16.
"""

import numpy as np
import ml_dtypes

from contextlib import ExitStack


# ---------------------------------------------------------------- config ---
class Cfg:
    M = 8                 # cores
    N_NODES = 100000
    N_EDGES = 1600000
    IN_DIM = 128
    HID = 64
    NCLS = 40
    SHARD = 12500         # avg real dst nodes per core
    NT = 98               # dst tiles per core (128 each)
    KSEGQ = (5, 5, 5, 5)  # chunks (of 128 edges) per (tile, quarter) segment,
    #                       per quarter; overwritten by the balancer readback
    SLABC = 10            # chunks per gather slab
    MSGBUFS = 10          # msg slab buffers (pipeline depth)
    PREGEN = 2            # slab generations prefetched ahead
    X_BF16 = True         # phase-A (x@W1) in bf16
    SINGLE_PACKET = False  # multi-packet gathers (single-packet hangs >~1K idxs)
    NQUEUES = 4           # spread gathers over all 4 SWDGE queues

    @property
    def PADSHARD(self):
        return self.NT * 128

    @property
    def NPAD(self):
        return self.PADSHARD * self.M

    @property
    def QROWS(self):
        return self.NPAD // 4

    @property
    def SEGQ(self):
        return [k * 128 for k in self.KSEGQ]

    @property
    def CQQ(self):
        return [self.NT * k for k in self.KSEGQ]   # chunks per quarter

    @property
    def QCOFF(self):
        # chunk offset of each quarter in the global stream
        off, out = 0, []
        for cq in self.CQQ:
            out.append(off)
            off += cq
        return out

    @property
    def NSLABQ(self):
        for cq in self.CQQ:
            assert cq % self.SLABC == 0, (cq, self.SLABC)
        return [cq // self.SLABC for cq in self.CQQ]

    @property
    def CHUNKS(self):
        return sum(self.CQQ)

    @property
    def ASLAB(self):
        # phase-A node slab: 1792 own-shard nodes (14 x 128)
        assert self.PADSHARD % 1792 == 0
        return self.PADSHARD // 1792


CFG = Cfg()


# ------------------------------------------------------------- host plan ---
def _plan(cfg, edge_row, edge_col, edge_val):
    """Bucket/sort/pad edges per core. Returns per-core arrays:
    idx16 [128, 4*CQ*128/16] int16, ldstT [128, CHUNKS] f32, valT [128, CHUNKS] f32.
    """
    M, SHARD, PADSHARD = cfg.M, cfg.SHARD, cfg.PADSHARD
    NT, KSEG, SEG, CQ, QROWS = cfg.NT, cfg.KSEG, cfg.SEG, cfg.CQ, cfg.QROWS

    # padded (table) node id and quarter decomposition of sources
    psrc = (edge_col // SHARD) * PADSHARD + (edge_col % SHARD)
    q_of = psrc // QROWS
    i_of = psrc % QROWS
    core_of = edge_row // SHARD
    dloc = edge_row % SHARD
    t_of = dloc // 128
    l_of = dloc % 128

    L = 4 * CQ * 128
    idx_all, ldst_all, val_all = [], [], []
    for c in range(M):
        sel = core_of == c
        # order: (quarter, tile) segment id
        segid = q_of[sel] * NT + t_of[sel]
        order = np.argsort(segid, kind="stable")
        sid = segid[order]
        idx_s = i_of[sel][order]
        l_s = l_of[sel][order]
        v_s = edge_val[sel][order]

        counts = np.bincount(sid, minlength=4 * NT)
        if counts.max() > SEG:
            raise ValueError(f"segment overflow: {counts.max()} > {SEG}")
        # place into padded stream: segment s at offset s*SEG
        starts = np.arange(4 * NT) * SEG
        pos = starts[sid] + (np.arange(sid.size) - np.concatenate(([0], np.cumsum(counts)))[sid])

        idx = np.zeros(L, dtype=np.int16)
        ldst = np.zeros(L, dtype=np.float32)
        val = np.zeros(L, dtype=np.float32)
        idx[pos] = idx_s.astype(np.int16)
        ldst[pos] = l_s.astype(np.float32)
        val[pos] = v_s.astype(np.float32)

        # wrap indices: idx i -> [i%16, i//16], replicated on all 8 q7 cores
        idxw = np.tile(idx.reshape(-1, 16).T, (8, 1)).copy()          # [128, L/16]
        ldstT = np.ascontiguousarray(ldst.reshape(-1, 128).T)        # [128, CHUNKS]
        valT = np.ascontiguousarray(val.reshape(-1, 128).T)
        idx_all.append(idxw)
        ldst_all.append(ldstT)
        val_all.append(valT)
    return idx_all, ldst_all, val_all


def _pack_x(cfg, x):
    """x [N, IN] -> per-core padded transposed shards [IN, PADSHARD]."""
    shards = []
    for c in range(cfg.M):
        xp = np.zeros((cfg.PADSHARD, cfg.IN_DIM), dtype=np.float32)
        xp[: cfg.SHARD] = x[c * cfg.SHARD:(c + 1) * cfg.SHARD]
        xT = np.ascontiguousarray(xp.T)
        if cfg.X_BF16:
            xT = xT.astype(ml_dtypes.bfloat16)
        shards.append(xT)
    return shards


# --------------------------------------------------------- device program ---
def _build(cfg, timing=False):
    from concourse import bacc, tile
    import concourse.mybir as mybir

    f32 = mybir.dt.float32
    bf16 = mybir.dt.bfloat16
    i16 = mybir.dt.int16
    AOP = mybir.AluOpType
    ACT = mybir.ActivationFunctionType

    xdt = bf16 if cfg.X_BF16 else f32

    nc = bacc.Bacc("TRN2", target_bir_lowering=False, debug=False,
                   num_devices=1 if timing else cfg.M,
                   dynamic_dma_scratch_size=getattr(cfg, "DMA_SCRATCH", 16384),
                   num_swdge_queues=getattr(cfg, "NQUEUES", 1))

    NPAD, QROWS, NT, KSEG, CQ, SLABC, NSLAB = (
        cfg.NPAD, cfg.QROWS, cfg.NT, cfg.KSEG, cfg.CQ, cfg.SLABC, cfg.NSLAB)
    CHUNKS, HID, NCLS, IN_DIM = cfg.CHUNKS, cfg.HID, cfg.NCLS, cfg.IN_DIM
    LQ16 = CQ * 128 // 16              # idx columns per quarter
    SLAB16 = SLABC * 128 // 16         # idx columns per slab
    NA = cfg.ASLAB                     # phase-A slabs (2048 nodes each)

    # -------- I/O
    XT = nc.dram_tensor("xt", [IN_DIM, cfg.PADSHARD], xdt, kind="ExternalInput")
    IDX = nc.dram_tensor("idx", [128, 4 * LQ16], i16, kind="ExternalInput")
    LDST = nc.dram_tensor("ldst", [128, CHUNKS], f32, kind="ExternalInput")
    VAL = nc.dram_tensor("val", [128, CHUNKS], f32, kind="ExternalInput")
    W1 = nc.dram_tensor("w1", [IN_DIM, HID], xdt, kind="ExternalInput")
    W2 = nc.dram_tensor("w2", [HID, HID], f32, kind="ExternalInput")
    WC = nc.dram_tensor("wc", [HID, NCLS], f32, kind="ExternalInput")
    B1 = nc.dram_tensor("b1", [128, HID], f32, kind="ExternalInput")   # replicated
    B2 = nc.dram_tensor("b2", [128, HID], f32, kind="ExternalInput")
    BC = nc.dram_tensor("bc", [128, NCLS], f32, kind="ExternalInput")
    IOTA = nc.dram_tensor("iota", [128, 128], bf16, kind="ExternalInput")
    IDENT = nc.dram_tensor("ident", [128, 128], f32, kind="ExternalInput")
    # transposed layout: OUT[p, t*NCLS+c] = node (t*128+p) class c
    OUT = nc.dram_tensor("out", [128, NT * NCLS], f32, kind="ExternalOutput")

    # -------- internal DRAM
    T1S = nc.dram_tensor("t1shard", [cfg.PADSHARD, 128], bf16)      # cols 64: junk
    T1F = nc.dram_tensor("t1full", [NPAD, 128], bf16, addr_space="Shared")
    T2S = nc.dram_tensor("t2shard", [cfg.PADSHARD, 128], bf16)
    T2F = nc.dram_tensor("t2full", [NPAD, 128], bf16, addr_space="Shared")

    with tile.TileContext(nc) as tc, ExitStack() as top:
        cpool = top.enter_context(tc.tile_pool(name="consts", bufs=1))
        w1s = cpool.tile([IN_DIM, HID], xdt)
        nc.sync.dma_start(out=w1s, in_=W1[:, :])
        w2s = cpool.tile([HID, HID], f32)
        nc.sync.dma_start(out=w2s, in_=W2[:, :])
        wcs = cpool.tile([HID, NCLS], f32)
        nc.sync.dma_start(out=wcs, in_=WC[:, :])
        b1s = cpool.tile([128, HID], f32)
        nc.sync.dma_start(out=b1s, in_=B1[:, :])
        b2s = cpool.tile([128, HID], f32)
        nc.sync.dma_start(out=b2s, in_=B2[:, :])
        bcs = cpool.tile([128, NCLS], f32)
        nc.sync.dma_start(out=bcs, in_=BC[:, :])
        b17s = cpool.tile([128, 7, HID], f32)
        for r in range(7):
            nc.sync.dma_start(out=b17s[:, r, :], in_=B1[:, :])
        iot = cpool.tile([128, 128], bf16)
        nc.sync.dma_start(out=iot, in_=IOTA[:, :])
        idn = cpool.tile([128, 128], f32)
        nc.sync.dma_start(out=idn, in_=IDENT[:, :])

        edg = top.enter_context(tc.tile_pool(name="edg", bufs=1))
        ldsts = edg.tile([128, CHUNKS], f32)
        nc.sync.dma_start(out=ldsts, in_=LDST[:, :])
        vals = edg.tile([128, CHUNKS], f32)
        nc.sync.dma_start(out=vals, in_=VAL[:, :])
        # persistent per-quarter edge-gather indices (used by both layers)
        its = []
        for q in range(4):
            it = edg.tile([128, LQ16], i16, tag=f"idx{q}")
            nc.sync.dma_start(out=it, in_=IDX[:, q * LQ16:(q + 1) * LQ16])
            its.append(it)

        accp = top.enter_context(tc.tile_pool(name="acc", bufs=1))

        # ====== phase A: T1S = x_shard @ W1 + b1 (node-major bf16 rows),
        # then AllGather into the full table T1F.
        with tc.tile_pool(name="xa", bufs=3) as xa, \
             tc.tile_pool(name="sta", bufs=3) as sta, \
             tc.tile_pool(name="psa", bufs=4, space="PSUM") as psa:
            for s in range(NA):
                xs = xa.tile([128, 1792], xdt)
                nc.sync.dma_start(out=xs, in_=XT[:, s * 1792:(s + 1) * 1792])
                st = sta.tile([128, 14, HID], bf16)
                for h in range(2):
                    pb = psa.tile([128, 7, HID], f32)
                    for k7 in range(7):
                        k = h * 7 + k7
                        nc.tensor.matmul(pb[:, k7, :],
                                         lhsT=xs[:, k * 128:(k + 1) * 128],
                                         rhs=w1s, start=True, stop=True)
                    nc.vector.tensor_tensor(st[:, h * 7:(h + 1) * 7, :], pb,
                                            b17s, AOP.add)
                dst = T1S[s * 1792:(s + 1) * 1792, 0:HID].rearrange(
                    "(k p) f -> p k f", p=128)
                nc.sync.dma_start(out=dst, in_=st)
        if not timing:
            nc.gpsimd.collective_compute(
                "AllGather", mybir.AluOpType.bypass,
                replica_groups=[list(range(cfg.M))],
                ins=[T1S[:, :]], outs=[T1F[:, :]])

        # ============ spmm layer runner: per-tile single psum group across
        # all 4 quarters (slabs for all quarters retire in lockstep), with a
        # fused per-tile epilogue.
        def spmm_layer(tab, epilogue, pools):
            msg, vp, psb = pools
            slabs = [[None] * NSLAB for _ in range(4)]

            def ensure_slab(q, s):
                if s >= NSLAB:
                    return None
                if slabs[q][s] is None:
                    mt = msg.tile([128, SLABC, 128], bf16)
                    nc.gpsimd.dma_gather(
                        mt, tab[q * QROWS:(q + 1) * QROWS, :],
                        its[q][:, s * SLAB16:(s + 1) * SLAB16],
                        num_idxs=SLABC * 128, num_idxs_reg=SLABC * 128,
                        elem_size=128, elem_step=128,
                        single_packet=getattr(cfg, "SINGLE_PACKET", True),
                        queue_num=(q * NSLAB + s) % getattr(cfg, "NQUEUES", 1))
                    slabs[q][s] = mt
                return slabs[q][s]

            # prefetch: keep PREGEN slab generations in flight ahead of use
            PREGEN = getattr(cfg, "PREGEN", 2)
            for g in range(PREGEN):
                for q in range(4):
                    ensure_slab(q, g)

            for t in range(NT):
                s_now = t * KSEG // SLABC
                for q in range(4):
                    ensure_slab(q, s_now + PREGEN)
                ps = psb.tile([128, HID], f32)
                for q in range(4):
                    for k in range(KSEG):
                        j = t * KSEG + k                 # chunk in quarter
                        gj = q * CQ + j                  # global chunk
                        v = vp.tile([128, 128], bf16)
                        nc.vector.tensor_scalar(
                            v, iot, ldsts[:, gj:gj + 1], vals[:, gj:gj + 1],
                            AOP.is_equal, AOP.mult)
                        mt = ensure_slab(q, j // SLABC)
                        nc.tensor.matmul(ps, lhsT=v,
                                         rhs=mt[:, j % SLABC, 0:HID],
                                         start=(q == 0 and k == 0),
                                         stop=(q == 3 and k == KSEG - 1))
                epilogue(t, ps)

        # ================= layer 1 + phase C fused: T2S = relu(h1)@W2+b2
        for _rep in range(getattr(cfg, "REPS", 1)):
            _run_layers(cfg, nc, tc, tile, mybir, timing, accp, locals())
    nc.compile()
    return nc


def _run_layers(cfg, nc, tc, tile, mybir, timing, accp, env):
    f32 = mybir.dt.float32
    bf16 = mybir.dt.bfloat16
    i16 = mybir.dt.int16
    AOP = mybir.AluOpType
    ACT = mybir.ActivationFunctionType
    NPAD, QROWS, NT, KSEG, CQ, SLABC, NSLAB = (
        cfg.NPAD, cfg.QROWS, cfg.NT, cfg.KSEG, cfg.CQ, cfg.SLABC, cfg.NSLAB)
    CHUNKS, HID, NCLS, IN_DIM = cfg.CHUNKS, cfg.HID, cfg.NCLS, cfg.IN_DIM
    LQ16 = CQ * 128 // 16
    SLAB16 = SLABC * 128 // 16
    (T1F, T2S, T2F, IDX, OUT, iot, idn, ldsts, vals, w2s, wcs, b2s, bcs) = (
        env["T1F"], env["T2S"], env["T2F"], env["IDX"], env["OUT"], env["iot"],
        env["idn"], env["ldsts"], env["vals"], env["w2s"], env["wcs"],
        env["b2s"], env["bcs"])
    spmm_layer = env["spmm_layer"]

    if True:
        with tc.tile_pool(name="msg", bufs=getattr(cfg, "MSGBUFS", 8)) as msg, \
             tc.tile_pool(name="vp", bufs=8) as vp, \
             tc.tile_pool(name="psb", bufs=3, space="PSUM") as psb, \
             tc.tile_pool(name="tc1", bufs=3) as tp1, \
             tc.tile_pool(name="tc2", bufs=3) as tp2, \
             tc.tile_pool(name="tc3", bufs=3) as tp3, \
             tc.tile_pool(name="pst", bufs=2, space="PSUM") as pst, \
             tc.tile_pool(name="psc", bufs=2, space="PSUM") as psc:

            def epi1(t, ps):
                h1r = tp1.tile([128, HID], f32)
                nc.scalar.activation(h1r, ps, ACT.Relu)
                ptr = pst.tile([HID, 128], f32)
                nc.tensor.transpose(ptr, h1r, idn)
                h1t = tp2.tile([HID, 128], f32)
                nc.vector.tensor_copy(out=h1t, in_=ptr)
                ps2 = psc.tile([128, HID], f32)
                nc.tensor.matmul(ps2, lhsT=h1t, rhs=w2s, start=True, stop=True)
                t2t = tp3.tile([128, HID], bf16)
                nc.vector.tensor_tensor(t2t, ps2, b2s, AOP.add)
                nc.sync.dma_start(out=T2S[t * 128:(t + 1) * 128, 0:HID], in_=t2t)

            spmm_layer(T1F, epi1, (msg, vp, psb))
            if not timing:
                nc.gpsimd.collective_compute(
                    "AllGather", mybir.AluOpType.bypass,
                    replica_groups=[list(range(cfg.M))],
                    ins=[T2S[:, :]], outs=[T2F[:, :]])

        # ================= layer 2 + phase E fused: logits + log_softmax
        with tc.tile_pool(name="msg2", bufs=getattr(cfg, "MSGBUFS", 8)) as msg2, \
             tc.tile_pool(name="vp2", bufs=8) as vp2, \
             tc.tile_pool(name="psb2", bufs=3, space="PSUM") as psb2, \
             tc.tile_pool(name="te1", bufs=3) as te1, \
             tc.tile_pool(name="te2", bufs=3) as te2, \
             tc.tile_pool(name="pse", bufs=2, space="PSUM") as pse, \
             tc.tile_pool(name="psf", bufs=2, space="PSUM") as psf:
            lgacc = accp.tile([128, NT, NCLS], f32, tag="lgacc")
            negmacc = accp.tile([128, NT], f32, tag="negmacc")
            smacc = accp.tile([128, NT], f32, tag="smacc")

            lnacc = accp.tile([128, NT], f32, tag="lnacc")
            shacc = accp.tile([128, NT], f32, tag="shacc")
            FBLK = 14                      # tiles per finalize block

            def epi2(t, ps):
                h2s = te1.tile([128, HID], f32)
                nc.scalar.activation(h2s, ps, ACT.Copy)
                ptr = pse.tile([HID, 128], f32)
                nc.tensor.transpose(ptr, h2s, idn)
                h2t = te2.tile([HID, 128], f32)
                nc.vector.tensor_copy(out=h2t, in_=ptr)
                psl = psf.tile([128, NCLS], f32)
                nc.tensor.matmul(psl, lhsT=h2t, rhs=wcs, start=True, stop=True)
                nc.vector.tensor_tensor(lgacc[:, t, :], psl, bcs, AOP.add)
                nc.vector.tensor_reduce(negmacc[:, t:t + 1], lgacc[:, t, :],
                                        mybir.AxisListType.X, AOP.max,
                                        negate=True)
                et = te1.tile([128, NCLS], f32, tag="et")
                nc.scalar.activation(et, lgacc[:, t, :], ACT.Exp,
                                     bias=negmacc[:, t:t + 1],
                                     accum_out=smacc[:, t:t + 1])
                if (t + 1) % FBLK == 0:
                    # finalize block: log_softmax shift + store
                    b = t + 1 - FBLK
                    nc.scalar.activation(lnacc[:, b:t + 1], smacc[:, b:t + 1],
                                         ACT.Ln)
                    nc.vector.tensor_tensor(shacc[:, b:t + 1],
                                            lnacc[:, b:t + 1],
                                            negmacc[:, b:t + 1], AOP.subtract)
                    for u in range(b, t + 1):
                        nc.vector.tensor_scalar(lgacc[:, u, :], lgacc[:, u, :],
                                                shacc[:, u:u + 1], None,
                                                AOP.subtract)
                    nc.sync.dma_start(
                        out=OUT[:, b * NCLS:(t + 1) * NCLS],
                        in_=lgacc[:, b:t + 1, :])

            spmm_layer(T2F, epi2, (msg2, vp2, psb2))

    nc.compile()
    return nc


_NC_CACHE = {}


def _get_nc(cfg):
    key = (cfg.KSEG, cfg.X_BF16, getattr(cfg, "REPS", 1), cfg.SLABC)
    if key not in _NC_CACHE:
        _NC_CACHE[key] = _build(cfg)
    return _NC_CACHE[key]


# ------------------------------------------------------------------ main ---
def kernel(x, edge_row, edge_col, edge_val, W1, b1, W2, b2, Wc, bc,
           _run_kwargs=None):
    from concourse.bass_utils import run_bass_kernel_spmd

    cfg = CFG
    x = np.asarray(x, dtype=np.float32)
    edge_row = np.asarray(edge_row, dtype=np.int64)
    edge_col = np.asarray(edge_col, dtype=np.int64)
    edge_val = np.asarray(edge_val, dtype=np.float32)
    W1 = np.asarray(W1, dtype=np.float32)
    W2 = np.asarray(W2, dtype=np.float32)
    Wc = np.asarray(Wc, dtype=np.float32)
    b1 = np.asarray(b1, dtype=np.float32)
    b2 = np.asarray(b2, dtype=np.float32)
    bc = np.asarray(bc, dtype=np.float32)

    try:
        idx_all, ldst_all, val_all = _plan(cfg, edge_row, edge_col, edge_val)
    except ValueError:
        cfg.KSEG += 1
        idx_all, ldst_all, val_all = _plan(cfg, edge_row, edge_col, edge_val)

    xT = _pack_x(cfg, x)
    w1h = W1.astype(ml_dtypes.bfloat16) if cfg.X_BF16 else W1
    iota = np.tile(np.arange(128, dtype=np.float32), (128, 1)).astype(
        ml_dtypes.bfloat16)
    ident = np.eye(128, dtype=np.float32)
    b1r = np.tile(b1, (128, 1)).astype(np.float32)
    b2r = np.tile(b2, (128, 1)).astype(np.float32)
    bcr = np.tile(bc, (128, 1)).astype(np.float32)

    nc = _get_nc(cfg)
    in_maps = []
    for c in range(cfg.M):
        in_maps.append({
            "xt": xT[c], "idx": idx_all[c], "ldst": ldst_all[c],
            "val": val_all[c], "w1": w1h, "w2": W2, "wc": Wc,
            "b1": b1r, "b2": b2r, "bc": bcr, "iota": iota, "ident": ident,
        })
    kw = dict(_run_kwargs or {})
    res = run_bass_kernel_spmd(nc, in_maps, core_ids=list(range(cfg.M)), **kw)
    outs = []
    for c in range(cfg.M):
        o = np.asarray(res.results[c]["out"])          # [128, NT*NCLS]
        o = o.reshape(128, cfg.NT, cfg.NCLS).transpose(1, 0, 2).reshape(
            cfg.PADSHARD, cfg.NCLS)
        outs.append(o[: cfg.SHARD])
    out = np.concatenate(outs, axis=0)
    kernel.last_results = res
    return out.astype(np.float32)



# revision 34
# speedup vs baseline: 1.5834x; 1.0844x over previous
"""GCN node classifier (2x spmm + classifier + log_softmax) on 8 trn2 cores.

Strategy: destination-node 1D sharding. Each core owns 12,500 dst nodes and
the edges pointing at them. Layer tables (x@W1+b1, relu(h1)@W2+b2) are
node-major bf16 rows in DRAM; per-edge source rows are fetched with GPSIMD
dma_gather (int16 indices, so the table is addressed in 4 quarter views).
The segment-sum is a tensor-engine matmul against per-chunk scatter matrices
V[e, dst_lane] = edge_val[e] built on DVE with (iota == ldst) * val.
Between layers the per-shard T2 table is AllGather'ed into a Shared DRAM
tensor. All accumulation is f32 (PSUM); only table values are bf16.
"""

import numpy as np
import ml_dtypes

from contextlib import ExitStack


# ---------------------------------------------------------------- config ---
class Cfg:
    M = 8                 # cores
    N_NODES = 100000
    N_EDGES = 1600000
    IN_DIM = 128
    HID = 64
    NCLS = 40
    SHARD = 12500         # avg real dst nodes per core
    NT = 98               # dst tiles per core (128 each)
    KSEGQ = (5, 5, 5, 5)  # chunks (of 128 edges) per (tile, quarter) segment,
    #                       per quarter; overwritten by the balancer readback
    SLABC = 14            # chunks per gather slab (divides 98*k)
    MSGBUFS = 10          # msg slab buffers (pipeline depth)
    PREGEN = 2            # slab generations prefetched ahead
    X_BF16 = True         # phase-A (x@W1) in bf16
    SINGLE_PACKET = False  # multi-packet gathers (single-packet hangs >~1K idxs)
    NQUEUES = 4           # spread gathers over all 4 SWDGE queues

    @property
    def PADSHARD(self):
        return self.NT * 128

    @property
    def NPAD(self):
        return self.PADSHARD * self.M

    @property
    def QROWS(self):
        return self.NPAD // 4

    @property
    def SEGQ(self):
        return [k * 128 for k in self.KSEGQ]

    @property
    def CQQ(self):
        return [self.NT * k for k in self.KSEGQ]   # chunks per quarter

    @property
    def QCOFF(self):
        # chunk offset of each quarter in the global stream
        off, out = 0, []
        for cq in self.CQQ:
            out.append(off)
            off += cq
        return out

    @property
    def NSLABQ(self):
        for cq in self.CQQ:
            assert cq % self.SLABC == 0, (cq, self.SLABC)
        return [cq // self.SLABC for cq in self.CQQ]

    @property
    def CHUNKS(self):
        return sum(self.CQQ)

    @property
    def ASLAB(self):
        # phase-A node slab: 1792 own-shard nodes (14 x 128)
        assert self.PADSHARD % 1792 == 0
        return self.PADSHARD // 1792


CFG = Cfg()


# ------------------------------------------------------------- host plan ---
def _balance(cfg, edge_row, edge_col):
    """Assign every node a (core, tile, lane) slot, used both as its dst
    position and as its table position (phase-A/table sharding == dst
    sharding, so both spmm layers share one edge stream).  Greedy LPT on the
    gather cells (core, tile, src-quarter): each node's placement adds its
    in-edges (by already-placed source quarter) to its own (core,tile) cell
    column and its out-edges to the placed dsts' cells at quarter core//2.
    Returns slot[u] (global padded slot id) and the per-quarter chunk budget
    read back from the achieved packing.
    """
    M, NT, PADSHARD = cfg.M, cfg.NT, cfg.PADSHARD
    N = cfg.N_NODES
    NCELL = M * NT

    indeg = np.bincount(edge_row, minlength=N)
    outdeg = np.bincount(edge_col, minlength=N)

    # CSR by dst (in-edges: sources) and by src (out-edges: dsts)
    o_in = np.argsort(edge_row, kind="stable")
    in_src = edge_col[o_in]
    in_start = np.searchsorted(edge_row[o_in], np.arange(N + 1))
    o_out = np.argsort(edge_col, kind="stable")
    out_dst = edge_row[o_out]
    out_start = np.searchsorted(edge_col[o_out], np.arange(N + 1))

    # expected cell loads: an edge counts 1.0 once both endpoints are
    # placed; while its src is unplaced it is spread 0.25 per quarter.
    L = np.zeros((NCELL, 4), np.float64)
    fill = np.zeros(NCELL, np.int32)        # nodes per tile
    cfill = np.zeros(M, np.int32)           # real nodes per core
    node_cell = np.full(N, -1, np.int32)    # assigned (c*NT+t) or -1
    CAP = np.array(getattr(cfg, "BAL_CAPS", (511, 511, 511, 511)), np.float64)

    order = np.argsort(-(indeg + outdeg), kind="stable")
    tile_core = np.repeat(np.arange(M), NT)  # cell -> core
    tile_q = tile_core >> 1
    coretot = np.zeros(M, np.float64)        # expected edges per dst core
    E_CORE = edge_row.size / M + 60.0
    T_CAP = float(CAP.sum())                 # tile total target
    BIG = 1e9

    for u in order:
        # u's full in-edge profile: exact for placed sources, 1/4 otherwise
        srcs = in_src[in_start[u]:in_start[u + 1]]
        sc = node_cell[srcs]
        placed = sc >= 0
        inprof = np.bincount(tile_q[sc[placed]], minlength=4).astype(np.float64)
        inprof += 0.25 * float((~placed).sum())
        deg = float(inprof.sum())
        # score1[cell]: worst fill ratio of own cell column after adding
        s1 = ((L + inprof) / CAP).max(axis=1)
        # tile-total and core-total pressure
        s1 = np.maximum(s1, (L.sum(axis=1) + deg) / T_CAP)
        s3 = (coretot + deg) / E_CORE
        # score2[qq]: worst ratio among placed out-dst cells if u joins qq
        dsts = out_dst[out_start[u]:out_start[u + 1]]
        dc = node_cell[dsts]
        dc = dc[dc >= 0]
        if dc.size:
            cells, mult = np.unique(dc, return_counts=True)
            s2 = ((L[cells] + 0.75 * mult[:, None]) / CAP).max(axis=0)
        else:
            cells = mult = None
            s2 = np.zeros(4)
        score = np.maximum(np.maximum(s1, s2[tile_q]), s3[tile_core])
        score += 1e-5 * fill                 # deterministic tie-break
        score[fill >= 128] = BIG
        score[cfill[tile_core] >= PADSHARD] = BIG
        cell = int(np.argmin(score))
        c = cell // NT
        node_cell[u] = cell
        fill[cell] += 1
        cfill[c] += 1
        coretot[c] += deg
        L[cell] += inprof
        if cells is not None:
            # u's quarter now known: firm up the 0.25-spread charges
            L[cells] -= 0.25 * mult[:, None]
            L[cells, c >> 1] += mult

    # ---- repair pass on exact loads: relocate light sources out of the
    # few cells that ended 1-2 edges over the 512 target.
    TGT = int(getattr(cfg, "BAL_TGT", 512))
    Lx = np.zeros((NCELL, 4), np.int64)
    np.add.at(Lx, (node_cell[edge_row], tile_q[node_cell[edge_col]]), 1)
    tiletot = Lx.sum(axis=1)
    deg_all = indeg + outdeg
    for _ in range(400):
        over = np.argwhere(Lx > TGT)
        if over.size == 0:
            break
        oc, oq = int(over[0][0]), int(over[0][1])
        e_sel = np.where((node_cell[edge_row] == oc) &
                         (tile_q[node_cell[edge_col]] == oq))[0]
        cands, cmult = np.unique(edge_col[e_sel], return_counts=True)
        corder = np.argsort(deg_all[cands], kind="stable")
        moved = False
        for ci in corder[:160]:
            u = int(cands[ci])
            srcs = in_src[in_start[u]:in_start[u + 1]]
            dsts = out_dst[out_start[u]:out_start[u + 1]]
            if np.any(srcs == u):
                continue                     # self-loop: updates would split
            inprof = np.bincount(tile_q[node_cell[srcs]],
                                 minlength=4).astype(np.int64)
            ocells, omult = np.unique(node_cell[dsts], return_counts=True)
            old_cell = int(node_cell[u])
            old_q = int(tile_core[old_cell]) >> 1
            udeg = int(indeg[u])
            for q2 in range(4):
                if q2 == old_q:
                    continue
                if np.any(Lx[ocells, q2] + omult > TGT):
                    continue
                cand_cells = np.arange(2 * q2 * NT, (2 * q2 + 2) * NT)
                ok = (np.all(Lx[cand_cells] + inprof[None, :] <= TGT, axis=1)
                      & (tiletot[cand_cells] + udeg <= 4 * TGT)
                      & (fill[cand_cells] < 128))
                okc = cand_cells[ok]
                if okc.size == 0:
                    continue
                new_cell = int(okc[np.argmin(tiletot[okc])])
                # apply the move
                Lx[old_cell] -= inprof
                Lx[new_cell] += inprof
                tiletot[old_cell] -= udeg
                tiletot[new_cell] += udeg
                Lx[ocells, old_q] -= omult
                Lx[ocells, q2] += omult
                fill[old_cell] -= 1
                fill[new_cell] += 1
                node_cell[u] = new_cell
                moved = True
                break
            if moved:
                break
        if not moved:
            break

    lane = np.zeros(N, np.int32)
    ordc = np.argsort(node_cell, kind="stable")
    cc = node_cell[ordc]
    lane[ordc] = np.arange(N) - np.concatenate(
        ([0], np.cumsum(np.bincount(cc, minlength=NCELL))))[cc]
    slot = (node_cell // NT) * PADSHARD + (node_cell % NT) * 128 + lane

    # readback exact integer loads -> per-quarter chunk budgets
    Lx = np.zeros((NCELL, 4), np.int64)
    np.add.at(Lx, (node_cell[edge_row], tile_q[node_cell[edge_col]]), 1)
    ksegq = tuple(int(max(1, -(-int(Lx[:, q].max()) // 128))) for q in range(4))
    return slot, ksegq


def _plan(cfg, edge_row, edge_col, edge_val, slot):
    """Bucket/sort/pad edges per core using balanced slots. Returns per-core
    arrays: idx16 [128, CHUNKS*8] int16, ldstT/valT [128, CHUNKS] f32."""
    M, PADSHARD = cfg.M, cfg.PADSHARD
    NT, QROWS = cfg.NT, cfg.QROWS
    SEGQ, CQQ, QCOFF = cfg.SEGQ, cfg.CQQ, cfg.QCOFF

    psrc = slot[edge_col]
    q_of = psrc // QROWS
    i_of = psrc % QROWS
    dslot = slot[edge_row]
    core_of = dslot // PADSHARD
    dloc = dslot % PADSHARD
    t_of = dloc // 128
    l_of = dloc % 128

    # per-(q,t) segment slot offsets in the padded stream
    segq_arr = np.array(SEGQ)
    seg_base = np.concatenate(([0], np.cumsum(NT * segq_arr)))[:4]
    seg_cap = segq_arr  # capacity per (q,t)

    L = cfg.CHUNKS * 128
    idx_all, ldst_all, val_all = [], [], []
    for c in range(M):
        sel = core_of == c
        segid = q_of[sel] * NT + t_of[sel]
        order = np.argsort(segid, kind="stable")
        sid = segid[order]
        idx_s = i_of[sel][order]
        l_s = l_of[sel][order]
        v_s = edge_val[sel][order]

        counts = np.bincount(sid, minlength=4 * NT)
        caps = np.repeat(seg_cap, NT)
        if np.any(counts > caps):
            bad = int((counts - caps).max())
            raise ValueError(f"segment overflow by {bad}")
        starts = (seg_base[:, None] +
                  np.arange(NT)[None, :] * segq_arr[:, None]).reshape(-1)
        pos = starts[sid] + (np.arange(sid.size) -
                             np.concatenate(([0], np.cumsum(counts)))[sid])

        idx = np.zeros(L, dtype=np.int16)
        ldst = np.zeros(L, dtype=np.float32)
        val = np.zeros(L, dtype=np.float32)
        idx[pos] = idx_s.astype(np.int16)
        ldst[pos] = l_s.astype(np.float32)
        val[pos] = v_s.astype(np.float32)

        # wrap indices: idx i -> [i%16, i//16], replicated on all 8 q7 cores
        idxw = np.tile(idx.reshape(-1, 16).T, (8, 1)).copy()          # [128, L/16]
        ldstT = np.ascontiguousarray(ldst.reshape(-1, 128).T)        # [128, CHUNKS]
        valT = np.ascontiguousarray(val.reshape(-1, 128).T)
        idx_all.append(idxw)
        ldst_all.append(ldstT)
        val_all.append(valT)
    return idx_all, ldst_all, val_all


def _pack_x(cfg, x, slot):
    """x [N, IN] -> per-core padded transposed shards [IN, PADSHARD],
    rows placed at their balanced slots."""
    xp = np.zeros((cfg.NPAD, cfg.IN_DIM), dtype=np.float32)
    xp[slot] = x
    shards = []
    for c in range(cfg.M):
        xT = np.ascontiguousarray(
            xp[c * cfg.PADSHARD:(c + 1) * cfg.PADSHARD].T)
        if cfg.X_BF16:
            xT = xT.astype(ml_dtypes.bfloat16)
        shards.append(xT)
    return shards


# --------------------------------------------------------- device program ---
def _build(cfg, timing=False):
    from concourse import bacc, tile
    import concourse.mybir as mybir

    f32 = mybir.dt.float32
    bf16 = mybir.dt.bfloat16
    i16 = mybir.dt.int16
    AOP = mybir.AluOpType
    ACT = mybir.ActivationFunctionType

    xdt = bf16 if cfg.X_BF16 else f32

    nc = bacc.Bacc("TRN2", target_bir_lowering=False, debug=False,
                   num_devices=1 if timing else cfg.M,
                   dynamic_dma_scratch_size=getattr(cfg, "DMA_SCRATCH", 16384),
                   num_swdge_queues=getattr(cfg, "NQUEUES", 1))

    NPAD, QROWS, NT, SLABC = cfg.NPAD, cfg.QROWS, cfg.NT, cfg.SLABC
    KSEGQ, CQQ, QCOFF, NSLABQ = cfg.KSEGQ, cfg.CQQ, cfg.QCOFF, cfg.NSLABQ
    CHUNKS, HID, NCLS, IN_DIM = cfg.CHUNKS, cfg.HID, cfg.NCLS, cfg.IN_DIM
    LQ16Q = [cq * 128 // 16 for cq in CQQ]   # idx columns per quarter
    QIOFF = [o * 128 // 16 for o in QCOFF]   # idx column offset per quarter
    SLAB16 = SLABC * 128 // 16               # idx columns per slab
    NA = cfg.ASLAB                     # phase-A slabs (1792 nodes each)

    # -------- I/O
    XT = nc.dram_tensor("xt", [IN_DIM, cfg.PADSHARD], xdt, kind="ExternalInput")
    IDX = nc.dram_tensor("idx", [128, CHUNKS * 8], i16, kind="ExternalInput")
    LDST = nc.dram_tensor("ldst", [128, CHUNKS], f32, kind="ExternalInput")
    VAL = nc.dram_tensor("val", [128, CHUNKS], f32, kind="ExternalInput")
    W1 = nc.dram_tensor("w1", [IN_DIM, HID], xdt, kind="ExternalInput")
    W2 = nc.dram_tensor("w2", [HID, HID], f32, kind="ExternalInput")
    WC = nc.dram_tensor("wc", [HID, NCLS], f32, kind="ExternalInput")
    B1 = nc.dram_tensor("b1", [128, HID], f32, kind="ExternalInput")   # replicated
    B2 = nc.dram_tensor("b2", [128, HID], f32, kind="ExternalInput")
    BC = nc.dram_tensor("bc", [128, NCLS], f32, kind="ExternalInput")
    IOTA = nc.dram_tensor("iota", [128, 128], bf16, kind="ExternalInput")
    IDENT = nc.dram_tensor("ident", [128, 128], f32, kind="ExternalInput")
    # transposed layout: OUT[p, t*NCLS+c] = node (t*128+p) class c
    OUT = nc.dram_tensor("out", [128, NT * NCLS], f32, kind="ExternalOutput")

    # -------- internal DRAM
    T1S = nc.dram_tensor("t1shard", [cfg.PADSHARD, 128], bf16)      # cols 64: junk
    T1F = nc.dram_tensor("t1full", [NPAD, 128], bf16, addr_space="Shared")
    T2S = nc.dram_tensor("t2shard", [cfg.PADSHARD, 128], bf16)
    T2F = nc.dram_tensor("t2full", [NPAD, 128], bf16, addr_space="Shared")

    with tile.TileContext(nc) as tc, ExitStack() as top:
        cpool = top.enter_context(tc.tile_pool(name="consts", bufs=1))
        w1s = cpool.tile([IN_DIM, HID], xdt)
        nc.sync.dma_start(out=w1s, in_=W1[:, :])
        w2s = cpool.tile([HID, HID], f32)
        nc.sync.dma_start(out=w2s, in_=W2[:, :])
        wcs = cpool.tile([HID, NCLS], f32)
        nc.sync.dma_start(out=wcs, in_=WC[:, :])
        b1s = cpool.tile([128, HID], f32)
        nc.sync.dma_start(out=b1s, in_=B1[:, :])
        b2s = cpool.tile([128, HID], f32)
        nc.sync.dma_start(out=b2s, in_=B2[:, :])
        bcs = cpool.tile([128, NCLS], f32)
        nc.sync.dma_start(out=bcs, in_=BC[:, :])
        b17s = cpool.tile([128, 7, HID], f32)
        for r in range(7):
            nc.sync.dma_start(out=b17s[:, r, :], in_=B1[:, :])
        iot = cpool.tile([128, 128], bf16)
        nc.sync.dma_start(out=iot, in_=IOTA[:, :])
        idn = cpool.tile([128, 128], f32)
        nc.sync.dma_start(out=idn, in_=IDENT[:, :])

        edg = top.enter_context(tc.tile_pool(name="edg", bufs=1))
        ldsts = edg.tile([128, CHUNKS], f32)
        nc.sync.dma_start(out=ldsts, in_=LDST[:, :])
        vals = edg.tile([128, CHUNKS], f32)
        nc.sync.dma_start(out=vals, in_=VAL[:, :])
        # persistent per-quarter edge-gather indices (used by both layers)
        its = []
        for q in range(4):
            it = edg.tile([128, LQ16Q[q]], i16, tag=f"idx{q}")
            nc.sync.dma_start(out=it, in_=IDX[:, QIOFF[q]:QIOFF[q] + LQ16Q[q]])
            its.append(it)

        accp = top.enter_context(tc.tile_pool(name="acc", bufs=1))

        # ====== phase A: T1S = x_shard @ W1 + b1 (node-major bf16 rows),
        # then AllGather into the full table T1F.
        with tc.tile_pool(name="xa", bufs=3) as xa, \
             tc.tile_pool(name="sta", bufs=3) as sta, \
             tc.tile_pool(name="psa", bufs=4, space="PSUM") as psa:
            for s in range(NA):
                xs = xa.tile([128, 1792], xdt)
                nc.sync.dma_start(out=xs, in_=XT[:, s * 1792:(s + 1) * 1792])
                st = sta.tile([128, 14, HID], bf16)
                for h in range(2):
                    pb = psa.tile([128, 7, HID], f32)
                    for k7 in range(7):
                        k = h * 7 + k7
                        nc.tensor.matmul(pb[:, k7, :],
                                         lhsT=xs[:, k * 128:(k + 1) * 128],
                                         rhs=w1s, start=True, stop=True)
                    nc.vector.tensor_tensor(st[:, h * 7:(h + 1) * 7, :], pb,
                                            b17s, AOP.add)
                dst = T1S[s * 1792:(s + 1) * 1792, 0:HID].rearrange(
                    "(k p) f -> p k f", p=128)
                nc.sync.dma_start(out=dst, in_=st)
        if not timing:
            nc.gpsimd.collective_compute(
                "AllGather", mybir.AluOpType.bypass,
                replica_groups=[list(range(cfg.M))],
                ins=[T1S[:, :]], outs=[T1F[:, :]])

        # ============ spmm layer runner: per-tile single psum group across
        # all 4 quarters (slabs for all quarters retire in lockstep), with a
        # fused per-tile epilogue.
        def spmm_layer(tab, epilogue, pools):
            msg, vp, psb = pools
            slabs = [[None] * NSLABQ[q] for q in range(4)]
            nslab_tot = sum(NSLABQ)

            def ensure_slab(q, s):
                if s >= NSLABQ[q]:
                    return None
                if slabs[q][s] is None:
                    mt = msg.tile([128, SLABC, 128], bf16)
                    nc.gpsimd.dma_gather(
                        mt, tab[q * QROWS:(q + 1) * QROWS, :],
                        its[q][:, s * SLAB16:(s + 1) * SLAB16],
                        num_idxs=SLABC * 128, num_idxs_reg=SLABC * 128,
                        elem_size=128, elem_step=128,
                        single_packet=getattr(cfg, "SINGLE_PACKET", True),
                        queue_num=(q * NSLABQ[0] + s) % getattr(cfg, "NQUEUES", 1))
                    slabs[q][s] = mt
                return slabs[q][s]

            # prefetch: keep PREGEN slab generations in flight ahead of use
            PREGEN = getattr(cfg, "PREGEN", 2)
            for g in range(PREGEN):
                for q in range(4):
                    ensure_slab(q, g)

            for t in range(NT):
                for q in range(4):
                    ensure_slab(q, t * KSEGQ[q] // SLABC + PREGEN)
                ps = psb.tile([128, HID], f32)
                for q in range(4):
                    for k in range(KSEGQ[q]):
                        j = t * KSEGQ[q] + k             # chunk in quarter
                        gj = QCOFF[q] + j                # global chunk
                        v = vp.tile([128, 128], bf16)
                        nc.vector.tensor_scalar(
                            v, iot, ldsts[:, gj:gj + 1], vals[:, gj:gj + 1],
                            AOP.is_equal, AOP.mult)
                        mt = ensure_slab(q, j // SLABC)
                        nc.tensor.matmul(ps, lhsT=v,
                                         rhs=mt[:, j % SLABC, 0:HID],
                                         start=(q == 0 and k == 0),
                                         stop=(q == 3 and k == KSEGQ[3] - 1))
                epilogue(t, ps)

        # ================= layer 1 + phase C fused: T2S = relu(h1)@W2+b2
        for _rep in range(getattr(cfg, "REPS", 1)):
            _run_layers(cfg, nc, tc, tile, mybir, timing, accp, locals())
    nc.compile()
    return nc


def _run_layers(cfg, nc, tc, tile, mybir, timing, accp, env):
    f32 = mybir.dt.float32
    bf16 = mybir.dt.bfloat16
    i16 = mybir.dt.int16
    AOP = mybir.AluOpType
    ACT = mybir.ActivationFunctionType
    NPAD, QROWS, NT, SLABC = cfg.NPAD, cfg.QROWS, cfg.NT, cfg.SLABC
    CHUNKS, HID, NCLS, IN_DIM = cfg.CHUNKS, cfg.HID, cfg.NCLS, cfg.IN_DIM
    (T1F, T2S, T2F, IDX, OUT, iot, idn, ldsts, vals, w2s, wcs, b2s, bcs) = (
        env["T1F"], env["T2S"], env["T2F"], env["IDX"], env["OUT"], env["iot"],
        env["idn"], env["ldsts"], env["vals"], env["w2s"], env["wcs"],
        env["b2s"], env["bcs"])
    spmm_layer = env["spmm_layer"]

    if True:
        with tc.tile_pool(name="msg", bufs=getattr(cfg, "MSGBUFS", 8)) as msg, \
             tc.tile_pool(name="vp", bufs=8) as vp, \
             tc.tile_pool(name="psb", bufs=3, space="PSUM") as psb, \
             tc.tile_pool(name="tc1", bufs=3) as tp1, \
             tc.tile_pool(name="tc2", bufs=3) as tp2, \
             tc.tile_pool(name="tc3", bufs=3) as tp3, \
             tc.tile_pool(name="pst", bufs=2, space="PSUM") as pst, \
             tc.tile_pool(name="psc", bufs=2, space="PSUM") as psc:

            def epi1(t, ps):
                h1r = tp1.tile([128, HID], f32)
                nc.scalar.activation(h1r, ps, ACT.Relu)
                ptr = pst.tile([HID, 128], f32)
                nc.tensor.transpose(ptr, h1r, idn)
                h1t = tp2.tile([HID, 128], f32)
                nc.vector.tensor_copy(out=h1t, in_=ptr)
                ps2 = psc.tile([128, HID], f32)
                nc.tensor.matmul(ps2, lhsT=h1t, rhs=w2s, start=True, stop=True)
                t2t = tp3.tile([128, HID], bf16)
                nc.vector.tensor_tensor(t2t, ps2, b2s, AOP.add)
                nc.sync.dma_start(out=T2S[t * 128:(t + 1) * 128, 0:HID], in_=t2t)

            spmm_layer(T1F, epi1, (msg, vp, psb))
            if not timing:
                nc.gpsimd.collective_compute(
                    "AllGather", mybir.AluOpType.bypass,
                    replica_groups=[list(range(cfg.M))],
                    ins=[T2S[:, :]], outs=[T2F[:, :]])

        # ================= layer 2 + phase E fused: logits + log_softmax
        with tc.tile_pool(name="msg2", bufs=getattr(cfg, "MSGBUFS", 8)) as msg2, \
             tc.tile_pool(name="vp2", bufs=8) as vp2, \
             tc.tile_pool(name="psb2", bufs=3, space="PSUM") as psb2, \
             tc.tile_pool(name="te1", bufs=3) as te1, \
             tc.tile_pool(name="te2", bufs=3) as te2, \
             tc.tile_pool(name="pse", bufs=2, space="PSUM") as pse, \
             tc.tile_pool(name="psf", bufs=2, space="PSUM") as psf:
            lgacc = accp.tile([128, NT, NCLS], f32, tag="lgacc")
            negmacc = accp.tile([128, NT], f32, tag="negmacc")
            smacc = accp.tile([128, NT], f32, tag="smacc")

            lnacc = accp.tile([128, NT], f32, tag="lnacc")
            shacc = accp.tile([128, NT], f32, tag="shacc")
            FBLK = 14                      # tiles per finalize block

            def epi2(t, ps):
                h2s = te1.tile([128, HID], f32)
                nc.scalar.activation(h2s, ps, ACT.Copy)
                ptr = pse.tile([HID, 128], f32)
                nc.tensor.transpose(ptr, h2s, idn)
                h2t = te2.tile([HID, 128], f32)
                nc.vector.tensor_copy(out=h2t, in_=ptr)
                psl = psf.tile([128, NCLS], f32)
                nc.tensor.matmul(psl, lhsT=h2t, rhs=wcs, start=True, stop=True)
                nc.vector.tensor_tensor(lgacc[:, t, :], psl, bcs, AOP.add)
                nc.vector.tensor_reduce(negmacc[:, t:t + 1], lgacc[:, t, :],
                                        mybir.AxisListType.X, AOP.max,
                                        negate=True)
                et = te1.tile([128, NCLS], f32, tag="et")
                nc.scalar.activation(et, lgacc[:, t, :], ACT.Exp,
                                     bias=negmacc[:, t:t + 1],
                                     accum_out=smacc[:, t:t + 1])
                if (t + 1) % FBLK == 0:
                    # finalize block: log_softmax shift + store
                    b = t + 1 - FBLK
                    nc.scalar.activation(lnacc[:, b:t + 1], smacc[:, b:t + 1],
                                         ACT.Ln)
                    nc.vector.tensor_tensor(shacc[:, b:t + 1],
                                            lnacc[:, b:t + 1],
                                            negmacc[:, b:t + 1], AOP.subtract)
                    for u in range(b, t + 1):
                        nc.vector.tensor_scalar(lgacc[:, u, :], lgacc[:, u, :],
                                                shacc[:, u:u + 1], None,
                                                AOP.subtract)
                    nc.sync.dma_start(
                        out=OUT[:, b * NCLS:(t + 1) * NCLS],
                        in_=lgacc[:, b:t + 1, :])

            spmm_layer(T2F, epi2, (msg2, vp2, psb2))

    nc.compile()
    return nc


_NC_CACHE = {}
_PLAN_CACHE = {}


def _get_nc(cfg):
    key = (cfg.KSEGQ, cfg.X_BF16, getattr(cfg, "REPS", 1), cfg.SLABC,
           cfg.MSGBUFS, cfg.PREGEN)
    if key not in _NC_CACHE:
        _NC_CACHE[key] = _build(cfg)
    return _NC_CACHE[key]


# ------------------------------------------------------------------ main ---
def kernel(x, edge_row, edge_col, edge_val, W1, b1, W2, b2, Wc, bc,
           _run_kwargs=None):
    from concourse.bass_utils import run_bass_kernel_spmd

    cfg = CFG
    x = np.asarray(x, dtype=np.float32)
    edge_row = np.asarray(edge_row, dtype=np.int64)
    edge_col = np.asarray(edge_col, dtype=np.int64)
    edge_val = np.asarray(edge_val, dtype=np.float32)
    W1 = np.asarray(W1, dtype=np.float32)
    W2 = np.asarray(W2, dtype=np.float32)
    Wc = np.asarray(Wc, dtype=np.float32)
    b1 = np.asarray(b1, dtype=np.float32)
    b2 = np.asarray(b2, dtype=np.float32)
    bc = np.asarray(bc, dtype=np.float32)

    import hashlib
    h = hashlib.md5()
    h.update(np.ascontiguousarray(edge_row).tobytes())
    h.update(np.ascontiguousarray(edge_col).tobytes())
    fp = h.hexdigest()
    if fp in _PLAN_CACHE:
        slot, ksegq = _PLAN_CACHE[fp]
    else:
        slot, ksegq = _balance(cfg, edge_row, edge_col)
        _PLAN_CACHE[fp] = (slot, ksegq)
    cfg.KSEGQ = ksegq

    idx_all, ldst_all, val_all = _plan(cfg, edge_row, edge_col, edge_val, slot)
    xT = _pack_x(cfg, x, slot)
    w1h = W1.astype(ml_dtypes.bfloat16) if cfg.X_BF16 else W1
    iota = np.tile(np.arange(128, dtype=np.float32), (128, 1)).astype(
        ml_dtypes.bfloat16)
    ident = np.eye(128, dtype=np.float32)
    b1r = np.tile(b1, (128, 1)).astype(np.float32)
    b2r = np.tile(b2, (128, 1)).astype(np.float32)
    bcr = np.tile(bc, (128, 1)).astype(np.float32)

    nc = _get_nc(cfg)
    in_maps = []
    for c in range(cfg.M):
        in_maps.append({
            "xt": xT[c], "idx": idx_all[c], "ldst": ldst_all[c],
            "val": val_all[c], "w1": w1h, "w2": W2, "wc": Wc,
            "b1": b1r, "b2": b2r, "bc": bcr, "iota": iota, "ident": ident,
        })
    kw = dict(_run_kwargs or {})
    res = run_bass_kernel_spmd(nc, in_maps, core_ids=list(range(cfg.M)), **kw)
    rows = []
    for c in range(cfg.M):
        o = np.asarray(res.results[c]["out"])          # [128, NT*NCLS]
        rows.append(o.reshape(128, cfg.NT, cfg.NCLS).transpose(1, 0, 2)
                    .reshape(cfg.PADSHARD, cfg.NCLS))
    allrows = np.concatenate(rows, axis=0)             # [NPAD, NCLS]
    out = allrows[slot]                                # unpermute to node order
    kernel.last_results = res
    return out.astype(np.float32)



# revision 38
# speedup vs baseline: 1.7565x; 1.1093x over previous
"""GCN node classifier (2x spmm + classifier + log_softmax) on 8 trn2 cores.

Strategy: destination-node 1D sharding. Each core owns 12,500 dst nodes and
the edges pointing at them. Layer tables (x@W1+b1, relu(h1)@W2+b2) are
node-major bf16 rows in DRAM; per-edge source rows are fetched with GPSIMD
dma_gather (int16 indices, so the table is addressed in 4 quarter views).
The segment-sum is a tensor-engine matmul against per-chunk scatter matrices
V[e, dst_lane] = edge_val[e] built on DVE with (iota == ldst) * val.
Between layers the per-shard T2 table is AllGather'ed into a Shared DRAM
tensor. All accumulation is f32 (PSUM); only table values are bf16.
"""

import numpy as np
import ml_dtypes

from contextlib import ExitStack


# ---------------------------------------------------------------- config ---
class Cfg:
    M = 8                 # cores
    N_NODES = 100000
    N_EDGES = 1600000
    IN_DIM = 128
    HID = 64
    NCLS = 40
    SHARD = 12500         # avg real dst nodes per core
    NT = 98               # dst tiles per core (128 each)
    # chunks (of 128 edges) per (tile, quarter) segment: full per-tile
    # budget table, overwritten by the balancer readback
    KSEGT = tuple((5, 5, 5, 5) for _ in range(98))
    NFAT = 8              # balancer: trailing tiles with a fat (640) target
    SLABC = 7             # chunks per gather slab
    MSGBUFS = 12          # msg slab buffers (pipeline depth)
    PREGEN = 2            # slab generations prefetched ahead
    X_BF16 = True         # phase-A (x@W1) in bf16
    SINGLE_PACKET = False  # multi-packet gathers (single-packet hangs >~1K idxs)
    NQUEUES = 4           # spread gathers over all 4 SWDGE queues

    @property
    def PADSHARD(self):
        return self.NT * 128

    @property
    def NPAD(self):
        return self.PADSHARD * self.M

    @property
    def QROWS(self):
        return self.NPAD // 4

    @property
    def CQQ(self):
        # chunks per quarter (sum over tiles)
        return [sum(kt[q] for kt in self.KSEGT) for q in range(4)]

    @property
    def CUMT(self):
        # CUMT[q][t]: chunks before tile t within quarter q
        out = []
        for q in range(4):
            off, col = 0, []
            for t in range(self.NT):
                col.append(off)
                off += self.KSEGT[t][q]
            out.append(col)
        return out

    @property
    def QCOFF(self):
        # chunk offset of each quarter in the global stream
        off, out = 0, []
        for cq in self.CQQ:
            out.append(off)
            off += cq
        return out

    @property
    def NSLABQ(self):
        # ceil: last slab of a quarter may be partial
        return [-(-cq // self.SLABC) for cq in self.CQQ]

    @property
    def CHUNKS(self):
        return sum(self.CQQ)

    @property
    def ASLAB(self):
        # phase-A node slab: 1792 own-shard nodes (14 x 128)
        assert self.PADSHARD % 1792 == 0
        return self.PADSHARD // 1792


CFG = Cfg()


# ------------------------------------------------------------- host plan ---
def _balance(cfg, edge_row, edge_col):
    """Assign every node a (core, tile, lane) slot, used both as its dst
    position and as its table position (phase-A/table sharding == dst
    sharding, so both spmm layers share one edge stream).  Greedy LPT on the
    gather cells (core, tile, src-quarter): each node's placement adds its
    in-edges (by already-placed source quarter) to its own (core,tile) cell
    column and its out-edges to the placed dsts' cells at quarter core//2.
    Returns slot[u] (global padded slot id) and the per-quarter chunk budget
    read back from the achieved packing.
    """
    M, NT, PADSHARD = cfg.M, cfg.NT, cfg.PADSHARD
    N = cfg.N_NODES
    NCELL = M * NT

    indeg = np.bincount(edge_row, minlength=N)
    outdeg = np.bincount(edge_col, minlength=N)

    # CSR by dst (in-edges: sources) and by src (out-edges: dsts)
    o_in = np.argsort(edge_row, kind="stable")
    in_src = edge_col[o_in]
    in_start = np.searchsorted(edge_row[o_in], np.arange(N + 1))
    o_out = np.argsort(edge_col, kind="stable")
    out_dst = edge_row[o_out]
    out_start = np.searchsorted(edge_col[o_out], np.arange(N + 1))

    # expected cell loads: an edge counts 1.0 once both endpoints are
    # placed; while its src is unplaced it is spread 0.25 per quarter.
    L = np.zeros((NCELL, 4), np.float64)
    fill = np.zeros(NCELL, np.int32)        # nodes per tile
    cfill = np.zeros(M, np.int32)           # real nodes per core
    node_cell = np.full(N, -1, np.int32)    # assigned (c*NT+t) or -1
    # per-cell caps: lean tiles target 511/column, trailing NFAT tiles 640
    NFAT = int(getattr(cfg, "NFAT", 8))
    tcap = np.full(NT, 511.0)
    tcap[NT - NFAT:] = 640.0
    CAP = np.tile(tcap[:, None], (M, 4))    # [NCELL, 4]

    order = np.argsort(-(indeg + outdeg), kind="stable")
    tile_core = np.repeat(np.arange(M), NT)  # cell -> core
    tile_q = tile_core >> 1
    coretot = np.zeros(M, np.float64)        # expected edges per dst core
    E_CORE = edge_row.size / M + 60.0
    T_CAP = CAP.sum(axis=1)                  # per-tile total target
    BIG = 1e9

    for u in order:
        # u's full in-edge profile: exact for placed sources, 1/4 otherwise
        srcs = in_src[in_start[u]:in_start[u + 1]]
        sc = node_cell[srcs]
        placed = sc >= 0
        inprof = np.bincount(tile_q[sc[placed]], minlength=4).astype(np.float64)
        inprof += 0.25 * float((~placed).sum())
        deg = float(inprof.sum())
        # score1[cell]: worst fill ratio of own cell column after adding
        s1 = ((L + inprof) / CAP).max(axis=1)
        # tile-total and core-total pressure
        s1 = np.maximum(s1, (L.sum(axis=1) + deg) / T_CAP)
        s3 = (coretot + deg) / E_CORE
        # score2[qq]: worst ratio among placed out-dst cells if u joins qq
        dsts = out_dst[out_start[u]:out_start[u + 1]]
        dc = node_cell[dsts]
        dc = dc[dc >= 0]
        if dc.size:
            cells, mult = np.unique(dc, return_counts=True)
            s2 = ((L[cells] + 0.75 * mult[:, None]) / CAP[cells]).max(axis=0)
        else:
            cells = mult = None
            s2 = np.zeros(4)
        score = np.maximum(np.maximum(s1, s2[tile_q]), s3[tile_core])
        score += 1e-5 * fill                 # deterministic tie-break
        score[fill >= 128] = BIG
        score[cfill[tile_core] >= PADSHARD] = BIG
        cell = int(np.argmin(score))
        c = cell // NT
        node_cell[u] = cell
        fill[cell] += 1
        cfill[c] += 1
        coretot[c] += deg
        L[cell] += inprof
        if cells is not None:
            # u's quarter now known: firm up the 0.25-spread charges
            L[cells] -= 0.25 * mult[:, None]
            L[cells, c >> 1] += mult

    # ---- repair pass on exact loads: relocate light sources out of the
    # few cells that ended 1-2 edges over the 512 target.
    TGTM = np.where(CAP > 600, 640, 512).astype(np.int64)  # [NCELL, 4]
    TTOT = TGTM.sum(axis=1)
    Lx = np.zeros((NCELL, 4), np.int64)
    np.add.at(Lx, (node_cell[edge_row], tile_q[node_cell[edge_col]]), 1)
    tiletot = Lx.sum(axis=1)
    deg_all = indeg + outdeg
    for _ in range(400):
        over = np.argwhere(Lx > TGTM)
        if over.size == 0:
            break
        oc, oq = int(over[0][0]), int(over[0][1])
        e_sel = np.where((node_cell[edge_row] == oc) &
                         (tile_q[node_cell[edge_col]] == oq))[0]
        cands, cmult = np.unique(edge_col[e_sel], return_counts=True)
        corder = np.argsort(deg_all[cands], kind="stable")
        moved = False
        for ci in corder[:160]:
            u = int(cands[ci])
            srcs = in_src[in_start[u]:in_start[u + 1]]
            dsts = out_dst[out_start[u]:out_start[u + 1]]
            if np.any(srcs == u):
                continue                     # self-loop: updates would split
            inprof = np.bincount(tile_q[node_cell[srcs]],
                                 minlength=4).astype(np.int64)
            ocells, omult = np.unique(node_cell[dsts], return_counts=True)
            old_cell = int(node_cell[u])
            old_q = int(tile_core[old_cell]) >> 1
            udeg = int(indeg[u])
            for q2 in range(4):
                if q2 == old_q:
                    continue
                if np.any(Lx[ocells, q2] + omult > TGTM[ocells, q2]):
                    continue
                cand_cells = np.arange(2 * q2 * NT, (2 * q2 + 2) * NT)
                ok = (np.all(Lx[cand_cells] + inprof[None, :]
                             <= TGTM[cand_cells], axis=1)
                      & (tiletot[cand_cells] + udeg <= TTOT[cand_cells])
                      & (fill[cand_cells] < 128))
                okc = cand_cells[ok]
                if okc.size == 0:
                    continue
                new_cell = int(okc[np.argmin(tiletot[okc])])
                # apply the move
                Lx[old_cell] -= inprof
                Lx[new_cell] += inprof
                tiletot[old_cell] -= udeg
                tiletot[new_cell] += udeg
                Lx[ocells, old_q] -= omult
                Lx[ocells, q2] += omult
                fill[old_cell] -= 1
                fill[new_cell] += 1
                node_cell[u] = new_cell
                moved = True
                break
            if moved:
                break
        if not moved:
            break

    lane = np.zeros(N, np.int32)
    ordc = np.argsort(node_cell, kind="stable")
    cc = node_cell[ordc]
    lane[ordc] = np.arange(N) - np.concatenate(
        ([0], np.cumsum(np.bincount(cc, minlength=NCELL))))[cc]
    slot = (node_cell // NT) * PADSHARD + (node_cell % NT) * 128 + lane

    # readback exact loads -> per-(tile, quarter) chunk budget table
    Lx = np.zeros((NCELL, 4), np.int64)
    np.add.at(Lx, (node_cell[edge_row], tile_q[node_cell[edge_col]]), 1)
    tmax = Lx.reshape(M, NT, 4).max(axis=0)          # [NT, 4]
    ksegt = tuple(tuple(int(max(1, -(-int(tmax[t, q]) // 128)))
                        for q in range(4)) for t in range(NT))
    return slot, ksegt


def _plan(cfg, edge_row, edge_col, edge_val, slot):
    """Bucket/sort/pad edges per core using balanced slots. Returns per-core
    arrays: idx16 [128, CHUNKS*8] int16, ldstT/valT [128, CHUNKS] f32."""
    M, PADSHARD = cfg.M, cfg.PADSHARD
    NT, QROWS = cfg.NT, cfg.QROWS
    KSEGT, CQQ, QCOFF, CUMT = cfg.KSEGT, cfg.CQQ, cfg.QCOFF, cfg.CUMT

    psrc = slot[edge_col]
    q_of = psrc // QROWS
    i_of = psrc % QROWS
    dslot = slot[edge_row]
    core_of = dslot // PADSHARD
    dloc = dslot % PADSHARD
    t_of = dloc // 128
    l_of = dloc % 128

    # per-(q,t) segment capacities and slot offsets in the padded stream
    ksegt_arr = np.array(KSEGT, np.int64)            # [NT, 4]
    seg_cap = (ksegt_arr.T * 128).reshape(-1)        # [(q,t)] capacity
    starts = ((np.array(QCOFF)[:, None] + np.array(CUMT)) * 128).reshape(-1)

    L = cfg.CHUNKS * 128
    idx_all, ldst_all, val_all = [], [], []
    for c in range(M):
        sel = core_of == c
        segid = q_of[sel] * NT + t_of[sel]
        order = np.argsort(segid, kind="stable")
        sid = segid[order]
        idx_s = i_of[sel][order]
        l_s = l_of[sel][order]
        v_s = edge_val[sel][order]

        counts = np.bincount(sid, minlength=4 * NT)
        if np.any(counts > seg_cap):
            bad = int((counts - seg_cap).max())
            raise ValueError(f"segment overflow by {bad}")
        pos = starts[sid] + (np.arange(sid.size) -
                             np.concatenate(([0], np.cumsum(counts)))[sid])

        idx = np.zeros(L, dtype=np.int16)
        ldst = np.zeros(L, dtype=np.float32)
        val = np.zeros(L, dtype=np.float32)
        idx[pos] = idx_s.astype(np.int16)
        ldst[pos] = l_s.astype(np.float32)
        val[pos] = v_s.astype(np.float32)

        # wrap indices: idx i -> [i%16, i//16], replicated on all 8 q7 cores
        idxw = np.tile(idx.reshape(-1, 16).T, (8, 1)).copy()          # [128, L/16]
        ldstT = np.ascontiguousarray(ldst.reshape(-1, 128).T)        # [128, CHUNKS]
        valT = np.ascontiguousarray(val.reshape(-1, 128).T)
        idx_all.append(idxw)
        ldst_all.append(ldstT)
        val_all.append(valT)
    return idx_all, ldst_all, val_all


def _pack_x(cfg, x, slot):
    """x [N, IN] -> per-core padded transposed shards [IN, PADSHARD],
    rows placed at their balanced slots."""
    xp = np.zeros((cfg.NPAD, cfg.IN_DIM), dtype=np.float32)
    xp[slot] = x
    shards = []
    for c in range(cfg.M):
        xT = np.ascontiguousarray(
            xp[c * cfg.PADSHARD:(c + 1) * cfg.PADSHARD].T)
        if cfg.X_BF16:
            xT = xT.astype(ml_dtypes.bfloat16)
        shards.append(xT)
    return shards


# --------------------------------------------------------- device program ---
def _build(cfg, timing=False):
    from concourse import bacc, tile
    import concourse.mybir as mybir

    f32 = mybir.dt.float32
    bf16 = mybir.dt.bfloat16
    i16 = mybir.dt.int16
    AOP = mybir.AluOpType
    ACT = mybir.ActivationFunctionType

    xdt = bf16 if cfg.X_BF16 else f32

    nc = bacc.Bacc("TRN2", target_bir_lowering=False, debug=False,
                   num_devices=1 if timing else cfg.M,
                   dynamic_dma_scratch_size=getattr(cfg, "DMA_SCRATCH", 16384),
                   num_swdge_queues=getattr(cfg, "NQUEUES", 1))

    NPAD, QROWS, NT, SLABC = cfg.NPAD, cfg.QROWS, cfg.NT, cfg.SLABC
    KSEGT, CQQ, QCOFF, NSLABQ = cfg.KSEGT, cfg.CQQ, cfg.QCOFF, cfg.NSLABQ
    CUMT = cfg.CUMT
    CHUNKS, HID, NCLS, IN_DIM = cfg.CHUNKS, cfg.HID, cfg.NCLS, cfg.IN_DIM
    LQ16Q = [cq * 128 // 16 for cq in CQQ]   # idx columns per quarter
    QIOFF = [o * 128 // 16 for o in QCOFF]   # idx column offset per quarter
    SLAB16 = SLABC * 128 // 16               # idx columns per slab
    NA = cfg.ASLAB                     # phase-A slabs (1792 nodes each)

    # -------- I/O
    XT = nc.dram_tensor("xt", [IN_DIM, cfg.PADSHARD], xdt, kind="ExternalInput")
    IDX = nc.dram_tensor("idx", [128, CHUNKS * 8], i16, kind="ExternalInput")
    LDST = nc.dram_tensor("ldst", [128, CHUNKS], f32, kind="ExternalInput")
    VAL = nc.dram_tensor("val", [128, CHUNKS], f32, kind="ExternalInput")
    W1 = nc.dram_tensor("w1", [IN_DIM, HID], xdt, kind="ExternalInput")
    W2 = nc.dram_tensor("w2", [HID, HID], f32, kind="ExternalInput")
    WC = nc.dram_tensor("wc", [HID, NCLS], f32, kind="ExternalInput")
    B1 = nc.dram_tensor("b1", [128, HID], f32, kind="ExternalInput")   # replicated
    B2 = nc.dram_tensor("b2", [128, HID], f32, kind="ExternalInput")
    BC = nc.dram_tensor("bc", [128, NCLS], f32, kind="ExternalInput")
    IOTA = nc.dram_tensor("iota", [128, 128], bf16, kind="ExternalInput")
    IDENT = nc.dram_tensor("ident", [128, 128], f32, kind="ExternalInput")
    # transposed layout: OUT[p, t*NCLS+c] = node (t*128+p) class c
    OUT = nc.dram_tensor("out", [128, NT * NCLS], f32, kind="ExternalOutput")

    # -------- internal DRAM
    T1S = nc.dram_tensor("t1shard", [cfg.PADSHARD, 128], bf16)      # cols 64: junk
    T1F = nc.dram_tensor("t1full", [NPAD, 128], bf16, addr_space="Shared")
    T2S = nc.dram_tensor("t2shard", [cfg.PADSHARD, 128], bf16)
    T2F = nc.dram_tensor("t2full", [NPAD, 128], bf16, addr_space="Shared")

    with tile.TileContext(nc) as tc, ExitStack() as top:
        cpool = top.enter_context(tc.tile_pool(name="consts", bufs=1))
        w1s = cpool.tile([IN_DIM, HID], xdt)
        nc.sync.dma_start(out=w1s, in_=W1[:, :])
        w2s = cpool.tile([HID, HID], f32)
        nc.sync.dma_start(out=w2s, in_=W2[:, :])
        wcs = cpool.tile([HID, NCLS], f32)
        nc.sync.dma_start(out=wcs, in_=WC[:, :])
        b1s = cpool.tile([128, HID], f32)
        nc.sync.dma_start(out=b1s, in_=B1[:, :])
        b2s = cpool.tile([128, HID], f32)
        nc.sync.dma_start(out=b2s, in_=B2[:, :])
        bcs = cpool.tile([128, NCLS], f32)
        nc.sync.dma_start(out=bcs, in_=BC[:, :])
        b17s = cpool.tile([128, 7, HID], f32)
        for r in range(7):
            nc.sync.dma_start(out=b17s[:, r, :], in_=B1[:, :])
        iot = cpool.tile([128, 128], bf16)
        nc.sync.dma_start(out=iot, in_=IOTA[:, :])
        idn = cpool.tile([128, 128], f32)
        nc.sync.dma_start(out=idn, in_=IDENT[:, :])

        edg = top.enter_context(tc.tile_pool(name="edg", bufs=1))
        ldsts = edg.tile([128, CHUNKS], f32)
        nc.sync.dma_start(out=ldsts, in_=LDST[:, :])
        vals = edg.tile([128, CHUNKS], f32)
        nc.sync.dma_start(out=vals, in_=VAL[:, :])
        # persistent per-quarter edge-gather indices (used by both layers)
        its = []
        for q in range(4):
            it = edg.tile([128, LQ16Q[q]], i16, tag=f"idx{q}")
            nc.sync.dma_start(out=it, in_=IDX[:, QIOFF[q]:QIOFF[q] + LQ16Q[q]])
            its.append(it)

        accp = top.enter_context(tc.tile_pool(name="acc", bufs=1))

        # ====== phase A: T1S = x_shard @ W1 + b1 (node-major bf16 rows),
        # then AllGather into the full table T1F.
        with tc.tile_pool(name="xa", bufs=3) as xa, \
             tc.tile_pool(name="sta", bufs=3) as sta, \
             tc.tile_pool(name="psa", bufs=4, space="PSUM") as psa:
            for s in range(NA):
                xs = xa.tile([128, 1792], xdt)
                nc.sync.dma_start(out=xs, in_=XT[:, s * 1792:(s + 1) * 1792])
                st = sta.tile([128, 14, HID], bf16)
                for h in range(2):
                    pb = psa.tile([128, 7, HID], f32)
                    for k7 in range(7):
                        k = h * 7 + k7
                        nc.tensor.matmul(pb[:, k7, :],
                                         lhsT=xs[:, k * 128:(k + 1) * 128],
                                         rhs=w1s, start=True, stop=True)
                    nc.vector.tensor_tensor(st[:, h * 7:(h + 1) * 7, :], pb,
                                            b17s, AOP.add)
                dst = T1S[s * 1792:(s + 1) * 1792, 0:HID].rearrange(
                    "(k p) f -> p k f", p=128)
                nc.sync.dma_start(out=dst, in_=st)
        if not timing:
            nc.gpsimd.collective_compute(
                "AllGather", mybir.AluOpType.bypass,
                replica_groups=[list(range(cfg.M))],
                ins=[T1S[:, :]], outs=[T1F[:, :]])

        # ============ spmm layer runner: per-tile single psum group across
        # all 4 quarters (slabs for all quarters retire in lockstep), with a
        # fused per-tile epilogue.
        def spmm_layer(tab, epilogue, pools):
            msg, vp, psb = pools
            slabs = [[None] * NSLABQ[q] for q in range(4)]
            nslab_tot = sum(NSLABQ)

            def ensure_slab(q, s):
                if s >= NSLABQ[q]:
                    return None
                if slabs[q][s] is None:
                    sc = min(SLABC, CQQ[q] - s * SLABC)   # last slab: partial
                    mt = msg.tile([128, SLABC, 128], bf16)
                    nc.gpsimd.dma_gather(
                        mt[:, 0:sc, :], tab[q * QROWS:(q + 1) * QROWS, :],
                        its[q][:, s * SLAB16:s * SLAB16 + sc * 8],
                        num_idxs=sc * 128, num_idxs_reg=sc * 128,
                        elem_size=128, elem_step=128,
                        single_packet=getattr(cfg, "SINGLE_PACKET", True),
                        queue_num=(q * NSLABQ[0] + s) % getattr(cfg, "NQUEUES", 1))
                    slabs[q][s] = mt
                return slabs[q][s]

            # prefetch: keep PREGEN slab generations in flight ahead of use
            PREGEN = getattr(cfg, "PREGEN", 2)
            for g in range(PREGEN):
                for q in range(4):
                    ensure_slab(q, g)

            for t in range(NT):
                for q in range(4):
                    ensure_slab(q, CUMT[q][t] // SLABC + PREGEN)
                ps = psb.tile([128, HID], f32)
                for q in range(4):
                    for k in range(KSEGT[t][q]):
                        j = CUMT[q][t] + k               # chunk in quarter
                        gj = QCOFF[q] + j                # global chunk
                        v = vp.tile([128, 128], bf16)
                        nc.vector.tensor_scalar(
                            v, iot, ldsts[:, gj:gj + 1], vals[:, gj:gj + 1],
                            AOP.is_equal, AOP.mult)
                        mt = ensure_slab(q, j // SLABC)
                        nc.tensor.matmul(ps, lhsT=v,
                                         rhs=mt[:, j % SLABC, 0:HID],
                                         start=(q == 0 and k == 0),
                                         stop=(q == 3 and
                                               k == KSEGT[t][3] - 1))
                epilogue(t, ps)

        # ================= layer 1 + phase C fused: T2S = relu(h1)@W2+b2
        for _rep in range(getattr(cfg, "REPS", 1)):
            _run_layers(cfg, nc, tc, tile, mybir, timing, accp, locals())
    nc.compile()
    return nc


def _run_layers(cfg, nc, tc, tile, mybir, timing, accp, env):
    f32 = mybir.dt.float32
    bf16 = mybir.dt.bfloat16
    i16 = mybir.dt.int16
    AOP = mybir.AluOpType
    ACT = mybir.ActivationFunctionType
    NPAD, QROWS, NT, SLABC = cfg.NPAD, cfg.QROWS, cfg.NT, cfg.SLABC
    CHUNKS, HID, NCLS, IN_DIM = cfg.CHUNKS, cfg.HID, cfg.NCLS, cfg.IN_DIM
    (T1F, T2S, T2F, IDX, OUT, iot, idn, ldsts, vals, w2s, wcs, b2s, bcs) = (
        env["T1F"], env["T2S"], env["T2F"], env["IDX"], env["OUT"], env["iot"],
        env["idn"], env["ldsts"], env["vals"], env["w2s"], env["wcs"],
        env["b2s"], env["bcs"])
    spmm_layer = env["spmm_layer"]

    if True:
        with tc.tile_pool(name="msg", bufs=getattr(cfg, "MSGBUFS", 8)) as msg, \
             tc.tile_pool(name="vp", bufs=8) as vp, \
             tc.tile_pool(name="psb", bufs=3, space="PSUM") as psb, \
             tc.tile_pool(name="tc1", bufs=3) as tp1, \
             tc.tile_pool(name="tc2", bufs=3) as tp2, \
             tc.tile_pool(name="tc3", bufs=3) as tp3, \
             tc.tile_pool(name="pst", bufs=2, space="PSUM") as pst, \
             tc.tile_pool(name="psc", bufs=2, space="PSUM") as psc:

            def epi1(t, ps):
                h1r = tp1.tile([128, HID], f32)
                nc.scalar.activation(h1r, ps, ACT.Relu)
                ptr = pst.tile([HID, 128], f32)
                nc.tensor.transpose(ptr, h1r, idn)
                h1t = tp2.tile([HID, 128], f32)
                nc.vector.tensor_copy(out=h1t, in_=ptr)
                ps2 = psc.tile([128, HID], f32)
                nc.tensor.matmul(ps2, lhsT=h1t, rhs=w2s, start=True, stop=True)
                t2t = tp3.tile([128, HID], bf16)
                nc.vector.tensor_tensor(t2t, ps2, b2s, AOP.add)
                nc.sync.dma_start(out=T2S[t * 128:(t + 1) * 128, 0:HID], in_=t2t)

            spmm_layer(T1F, epi1, (msg, vp, psb))
            if not timing:
                nc.gpsimd.collective_compute(
                    "AllGather", mybir.AluOpType.bypass,
                    replica_groups=[list(range(cfg.M))],
                    ins=[T2S[:, :]], outs=[T2F[:, :]])

        # ================= layer 2 + phase E fused: logits + log_softmax
        with tc.tile_pool(name="msg2", bufs=getattr(cfg, "MSGBUFS", 8)) as msg2, \
             tc.tile_pool(name="vp2", bufs=8) as vp2, \
             tc.tile_pool(name="psb2", bufs=3, space="PSUM") as psb2, \
             tc.tile_pool(name="te1", bufs=3) as te1, \
             tc.tile_pool(name="te2", bufs=3) as te2, \
             tc.tile_pool(name="pse", bufs=2, space="PSUM") as pse, \
             tc.tile_pool(name="psf", bufs=2, space="PSUM") as psf:
            lgacc = accp.tile([128, NT, NCLS], f32, tag="lgacc")
            negmacc = accp.tile([128, NT], f32, tag="negmacc")
            smacc = accp.tile([128, NT], f32, tag="smacc")

            lnacc = accp.tile([128, NT], f32, tag="lnacc")
            shacc = accp.tile([128, NT], f32, tag="shacc")
            FBLK = 14                      # tiles per finalize block

            def epi2(t, ps):
                h2s = te1.tile([128, HID], f32)
                nc.scalar.activation(h2s, ps, ACT.Copy)
                ptr = pse.tile([HID, 128], f32)
                nc.tensor.transpose(ptr, h2s, idn)
                h2t = te2.tile([HID, 128], f32)
                nc.vector.tensor_copy(out=h2t, in_=ptr)
                psl = psf.tile([128, NCLS], f32)
                nc.tensor.matmul(psl, lhsT=h2t, rhs=wcs, start=True, stop=True)
                nc.vector.tensor_tensor(lgacc[:, t, :], psl, bcs, AOP.add)
                nc.vector.tensor_reduce(negmacc[:, t:t + 1], lgacc[:, t, :],
                                        mybir.AxisListType.X, AOP.max,
                                        negate=True)
                et = te1.tile([128, NCLS], f32, tag="et")
                nc.scalar.activation(et, lgacc[:, t, :], ACT.Exp,
                                     bias=negmacc[:, t:t + 1],
                                     accum_out=smacc[:, t:t + 1])
                if (t + 1) % FBLK == 0:
                    # finalize block: log_softmax shift + store
                    b = t + 1 - FBLK
                    nc.scalar.activation(lnacc[:, b:t + 1], smacc[:, b:t + 1],
                                         ACT.Ln)
                    nc.vector.tensor_tensor(shacc[:, b:t + 1],
                                            lnacc[:, b:t + 1],
                                            negmacc[:, b:t + 1], AOP.subtract)
                    for u in range(b, t + 1):
                        nc.vector.tensor_scalar(lgacc[:, u, :], lgacc[:, u, :],
                                                shacc[:, u:u + 1], None,
                                                AOP.subtract)
                    nc.sync.dma_start(
                        out=OUT[:, b * NCLS:(t + 1) * NCLS],
                        in_=lgacc[:, b:t + 1, :])

            spmm_layer(T2F, epi2, (msg2, vp2, psb2))

    nc.compile()
    return nc


_NC_CACHE = {}
_PLAN_CACHE = {}


def _get_nc(cfg):
    key = (cfg.KSEGT, cfg.X_BF16, getattr(cfg, "REPS", 1), cfg.SLABC,
           cfg.MSGBUFS, cfg.PREGEN)
    if key not in _NC_CACHE:
        _NC_CACHE[key] = _build(cfg)
    return _NC_CACHE[key]


# ------------------------------------------------------------------ main ---
def kernel(x, edge_row, edge_col, edge_val, W1, b1, W2, b2, Wc, bc,
           _run_kwargs=None):
    from concourse.bass_utils import run_bass_kernel_spmd

    cfg = CFG
    x = np.asarray(x, dtype=np.float32)
    edge_row = np.asarray(edge_row, dtype=np.int64)
    edge_col = np.asarray(edge_col, dtype=np.int64)
    edge_val = np.asarray(edge_val, dtype=np.float32)
    W1 = np.asarray(W1, dtype=np.float32)
    W2 = np.asarray(W2, dtype=np.float32)
    Wc = np.asarray(Wc, dtype=np.float32)
    b1 = np.asarray(b1, dtype=np.float32)
    b2 = np.asarray(b2, dtype=np.float32)
    bc = np.asarray(bc, dtype=np.float32)

    import hashlib
    h = hashlib.md5()
    h.update(np.ascontiguousarray(edge_row).tobytes())
    h.update(np.ascontiguousarray(edge_col).tobytes())
    fp = h.hexdigest()
    if fp in _PLAN_CACHE:
        slot, ksegt = _PLAN_CACHE[fp]
    else:
        slot, ksegt = _balance(cfg, edge_row, edge_col)
        _PLAN_CACHE[fp] = (slot, ksegt)
    cfg.KSEGT = ksegt

    idx_all, ldst_all, val_all = _plan(cfg, edge_row, edge_col, edge_val, slot)
    xT = _pack_x(cfg, x, slot)
    w1h = W1.astype(ml_dtypes.bfloat16) if cfg.X_BF16 else W1
    iota = np.tile(np.arange(128, dtype=np.float32), (128, 1)).astype(
        ml_dtypes.bfloat16)
    ident = np.eye(128, dtype=np.float32)
    b1r = np.tile(b1, (128, 1)).astype(np.float32)
    b2r = np.tile(b2, (128, 1)).astype(np.float32)
    bcr = np.tile(bc, (128, 1)).astype(np.float32)

    nc = _get_nc(cfg)
    in_maps = []
    for c in range(cfg.M):
        in_maps.append({
            "xt": xT[c], "idx": idx_all[c], "ldst": ldst_all[c],
            "val": val_all[c], "w1": w1h, "w2": W2, "wc": Wc,
            "b1": b1r, "b2": b2r, "bc": bcr, "iota": iota, "ident": ident,
        })
    kw = dict(_run_kwargs or {})
    res = run_bass_kernel_spmd(nc, in_maps, core_ids=list(range(cfg.M)), **kw)
    rows = []
    for c in range(cfg.M):
        o = np.asarray(res.results[c]["out"])          # [128, NT*NCLS]
        rows.append(o.reshape(128, cfg.NT, cfg.NCLS).transpose(1, 0, 2)
                    .reshape(cfg.PADSHARD, cfg.NCLS))
    allrows = np.concatenate(rows, axis=0)             # [NPAD, NCLS]
    out = allrows[slot]                                # unpermute to node order
    kernel.last_results = res
    return out.astype(np.float32)



# revision 42
# speedup vs baseline: 1.7705x; 1.0080x over previous
"""GCN node classifier (2x spmm + classifier + log_softmax) on 8 trn2 cores.

Strategy: destination-node 1D sharding. Each core owns 12,500 dst nodes and
the edges pointing at them. Layer tables (x@W1+b1, relu(h1)@W2+b2) are
node-major bf16 rows in DRAM; per-edge source rows are fetched with GPSIMD
dma_gather (int16 indices, so the table is addressed in 4 quarter views).
The segment-sum is a tensor-engine matmul against per-chunk scatter matrices
V[e, dst_lane] = edge_val[e] built on DVE with (iota == ldst) * val.
Between layers the per-shard T2 table is AllGather'ed into a Shared DRAM
tensor. All accumulation is f32 (PSUM); only table values are bf16.
"""

import numpy as np
import ml_dtypes

from contextlib import ExitStack


# ---------------------------------------------------------------- config ---
class Cfg:
    M = 8                 # cores
    N_NODES = 100000
    N_EDGES = 1600000
    IN_DIM = 128
    HID = 64
    NCLS = 40
    SHARD = 12500         # avg real dst nodes per core
    NT = 98               # dst tiles per core (128 each)
    # chunks (of 128 edges) per (tile, quarter) segment: full per-tile
    # budget table, overwritten by the balancer readback
    KSEGT = tuple((5, 5, 5, 5) for _ in range(98))
    NFAT = 6              # balancer: trailing tiles with a fat (640) target
    SLABC = 8             # chunks per gather slab
    MSGBUFS = 12          # msg slab buffers (pipeline depth)
    PREGEN = 2            # slab generations prefetched ahead
    X_BF16 = True         # phase-A (x@W1) in bf16
    SINGLE_PACKET = False  # multi-packet gathers (single-packet hangs >~1K idxs)
    NQUEUES = 4           # spread gathers over all 4 SWDGE queues

    @property
    def PADSHARD(self):
        return self.NT * 128

    @property
    def NPAD(self):
        return self.PADSHARD * self.M

    @property
    def QROWS(self):
        return self.NPAD // 4

    @property
    def CQQ(self):
        # chunks per quarter (sum over tiles)
        return [sum(kt[q] for kt in self.KSEGT) for q in range(4)]

    @property
    def CUMT(self):
        # CUMT[q][t]: chunks before tile t within quarter q
        out = []
        for q in range(4):
            off, col = 0, []
            for t in range(self.NT):
                col.append(off)
                off += self.KSEGT[t][q]
            out.append(col)
        return out

    @property
    def QCOFF(self):
        # chunk offset of each quarter in the global stream
        off, out = 0, []
        for cq in self.CQQ:
            out.append(off)
            off += cq
        return out

    @property
    def NSLABQ(self):
        # ceil: last slab of a quarter may be partial
        return [-(-cq // self.SLABC) for cq in self.CQQ]

    @property
    def CHUNKS(self):
        return sum(self.CQQ)

    @property
    def ASLAB(self):
        # phase-A node slab: 1792 own-shard nodes (14 x 128)
        assert self.PADSHARD % 1792 == 0
        return self.PADSHARD // 1792


CFG = Cfg()


# ------------------------------------------------------------- host plan ---
def _balance(cfg, edge_row, edge_col):
    """Assign every node a (core, tile, lane) slot, used both as its dst
    position and as its table position (phase-A/table sharding == dst
    sharding, so both spmm layers share one edge stream).  Greedy LPT on the
    gather cells (core, tile, src-quarter): each node's placement adds its
    in-edges (by already-placed source quarter) to its own (core,tile) cell
    column and its out-edges to the placed dsts' cells at quarter core//2.
    Returns slot[u] (global padded slot id) and the per-quarter chunk budget
    read back from the achieved packing.
    """
    M, NT, PADSHARD = cfg.M, cfg.NT, cfg.PADSHARD
    N = cfg.N_NODES
    NCELL = M * NT

    indeg = np.bincount(edge_row, minlength=N)
    outdeg = np.bincount(edge_col, minlength=N)

    # CSR by dst (in-edges: sources) and by src (out-edges: dsts)
    o_in = np.argsort(edge_row, kind="stable")
    in_src = edge_col[o_in]
    in_start = np.searchsorted(edge_row[o_in], np.arange(N + 1))
    o_out = np.argsort(edge_col, kind="stable")
    out_dst = edge_row[o_out]
    out_start = np.searchsorted(edge_col[o_out], np.arange(N + 1))

    # expected cell loads: an edge counts 1.0 once both endpoints are
    # placed; while its src is unplaced it is spread 0.25 per quarter.
    L = np.zeros((NCELL, 4), np.float64)
    fill = np.zeros(NCELL, np.int32)        # nodes per tile
    cfill = np.zeros(M, np.int32)           # real nodes per core
    node_cell = np.full(N, -1, np.int32)    # assigned (c*NT+t) or -1
    # per-cell caps: lean tiles target 511/column, trailing NFAT tiles 640
    NFAT = int(getattr(cfg, "NFAT", 8))
    tcap = np.full(NT, 511.0)
    fat_idx = (np.arange(NFAT) * NT) // NFAT + NT // (2 * NFAT)
    if getattr(cfg, "FAT_SPREAD", False):
        tcap[fat_idx] = 640.0
    else:
        tcap[NT - NFAT:] = 640.0
    CAP = np.tile(tcap[:, None], (M, 4))    # [NCELL, 4]

    order = np.argsort(-(indeg + outdeg), kind="stable")
    tile_core = np.repeat(np.arange(M), NT)  # cell -> core
    tile_q = tile_core >> 1
    coretot = np.zeros(M, np.float64)        # expected edges per dst core
    E_CORE = edge_row.size / M + 60.0
    T_CAP = CAP.sum(axis=1)                  # per-tile total target
    BIG = 1e9

    for u in order:
        # u's full in-edge profile: exact for placed sources, 1/4 otherwise
        srcs = in_src[in_start[u]:in_start[u + 1]]
        sc = node_cell[srcs]
        placed = sc >= 0
        inprof = np.bincount(tile_q[sc[placed]], minlength=4).astype(np.float64)
        inprof += 0.25 * float((~placed).sum())
        deg = float(inprof.sum())
        # score1[cell]: worst fill ratio of own cell column after adding
        s1 = ((L + inprof) / CAP).max(axis=1)
        # tile-total and core-total pressure
        s1 = np.maximum(s1, (L.sum(axis=1) + deg) / T_CAP)
        s3 = (coretot + deg) / E_CORE
        # score2[qq]: worst ratio among placed out-dst cells if u joins qq
        dsts = out_dst[out_start[u]:out_start[u + 1]]
        dc = node_cell[dsts]
        dc = dc[dc >= 0]
        if dc.size:
            cells, mult = np.unique(dc, return_counts=True)
            s2 = ((L[cells] + 0.75 * mult[:, None]) / CAP[cells]).max(axis=0)
        else:
            cells = mult = None
            s2 = np.zeros(4)
        score = np.maximum(np.maximum(s1, s2[tile_q]), s3[tile_core])
        score += 1e-5 * fill                 # deterministic tie-break
        score[fill >= 128] = BIG
        score[cfill[tile_core] >= PADSHARD] = BIG
        cell = int(np.argmin(score))
        c = cell // NT
        node_cell[u] = cell
        fill[cell] += 1
        cfill[c] += 1
        coretot[c] += deg
        L[cell] += inprof
        if cells is not None:
            # u's quarter now known: firm up the 0.25-spread charges
            L[cells] -= 0.25 * mult[:, None]
            L[cells, c >> 1] += mult

    # ---- repair pass on exact loads: relocate light sources out of the
    # few cells that ended 1-2 edges over the 512 target.
    TGTM = np.where(CAP > 600, 640, 512).astype(np.int64)  # [NCELL, 4]
    TTOT = TGTM.sum(axis=1)
    Lx = np.zeros((NCELL, 4), np.int64)
    np.add.at(Lx, (node_cell[edge_row], tile_q[node_cell[edge_col]]), 1)
    tiletot = Lx.sum(axis=1)
    deg_all = indeg + outdeg
    for _ in range(400):
        over = np.argwhere(Lx > TGTM)
        if over.size == 0:
            break
        oc, oq = int(over[0][0]), int(over[0][1])
        e_sel = np.where((node_cell[edge_row] == oc) &
                         (tile_q[node_cell[edge_col]] == oq))[0]
        cands, cmult = np.unique(edge_col[e_sel], return_counts=True)
        corder = np.argsort(deg_all[cands], kind="stable")
        moved = False
        for ci in corder[:160]:
            u = int(cands[ci])
            srcs = in_src[in_start[u]:in_start[u + 1]]
            dsts = out_dst[out_start[u]:out_start[u + 1]]
            if np.any(srcs == u):
                continue                     # self-loop: updates would split
            inprof = np.bincount(tile_q[node_cell[srcs]],
                                 minlength=4).astype(np.int64)
            ocells, omult = np.unique(node_cell[dsts], return_counts=True)
            old_cell = int(node_cell[u])
            old_q = int(tile_core[old_cell]) >> 1
            udeg = int(indeg[u])
            for q2 in range(4):
                if q2 == old_q:
                    continue
                if np.any(Lx[ocells, q2] + omult > TGTM[ocells, q2]):
                    continue
                cand_cells = np.arange(2 * q2 * NT, (2 * q2 + 2) * NT)
                ok = (np.all(Lx[cand_cells] + inprof[None, :]
                             <= TGTM[cand_cells], axis=1)
                      & (tiletot[cand_cells] + udeg <= TTOT[cand_cells])
                      & (fill[cand_cells] < 128))
                okc = cand_cells[ok]
                if okc.size == 0:
                    continue
                new_cell = int(okc[np.argmin(tiletot[okc])])
                # apply the move
                Lx[old_cell] -= inprof
                Lx[new_cell] += inprof
                tiletot[old_cell] -= udeg
                tiletot[new_cell] += udeg
                Lx[ocells, old_q] -= omult
                Lx[ocells, q2] += omult
                fill[old_cell] -= 1
                fill[new_cell] += 1
                node_cell[u] = new_cell
                moved = True
                break
            if moved:
                break
        if not moved:
            break

    lane = np.zeros(N, np.int32)
    ordc = np.argsort(node_cell, kind="stable")
    cc = node_cell[ordc]
    lane[ordc] = np.arange(N) - np.concatenate(
        ([0], np.cumsum(np.bincount(cc, minlength=NCELL))))[cc]
    slot = (node_cell // NT) * PADSHARD + (node_cell % NT) * 128 + lane

    # readback exact loads -> per-(tile, quarter) chunk budget table
    Lx = np.zeros((NCELL, 4), np.int64)
    np.add.at(Lx, (node_cell[edge_row], tile_q[node_cell[edge_col]]), 1)
    tmax = Lx.reshape(M, NT, 4).max(axis=0)          # [NT, 4]
    ksegt = tuple(tuple(int(max(1, -(-int(tmax[t, q]) // 128)))
                        for q in range(4)) for t in range(NT))
    return slot, ksegt


def _plan(cfg, edge_row, edge_col, edge_val, slot):
    """Bucket/sort/pad edges per core using balanced slots. Returns per-core
    arrays: idx16 [128, CHUNKS*8] int16, ldstT/valT [128, CHUNKS] f32."""
    M, PADSHARD = cfg.M, cfg.PADSHARD
    NT, QROWS = cfg.NT, cfg.QROWS
    KSEGT, CQQ, QCOFF, CUMT = cfg.KSEGT, cfg.CQQ, cfg.QCOFF, cfg.CUMT

    psrc = slot[edge_col]
    q_of = psrc // QROWS
    i_of = psrc % QROWS
    dslot = slot[edge_row]
    core_of = dslot // PADSHARD
    dloc = dslot % PADSHARD
    t_of = dloc // 128
    l_of = dloc % 128

    # per-(q,t) segment capacities and slot offsets in the padded stream
    ksegt_arr = np.array(KSEGT, np.int64)            # [NT, 4]
    seg_cap = (ksegt_arr.T * 128).reshape(-1)        # [(q,t)] capacity
    starts = ((np.array(QCOFF)[:, None] + np.array(CUMT)) * 128).reshape(-1)

    L = cfg.CHUNKS * 128
    idx_all, ldst_all, val_all = [], [], []
    for c in range(M):
        sel = core_of == c
        segid = q_of[sel] * NT + t_of[sel]
        order = np.argsort(segid, kind="stable")
        sid = segid[order]
        idx_s = i_of[sel][order]
        l_s = l_of[sel][order]
        v_s = edge_val[sel][order]

        counts = np.bincount(sid, minlength=4 * NT)
        if np.any(counts > seg_cap):
            bad = int((counts - seg_cap).max())
            raise ValueError(f"segment overflow by {bad}")
        pos = starts[sid] + (np.arange(sid.size) -
                             np.concatenate(([0], np.cumsum(counts)))[sid])

        idx = np.zeros(L, dtype=np.int16)
        ldst = np.zeros(L, dtype=np.float32)
        val = np.zeros(L, dtype=np.float32)
        idx[pos] = idx_s.astype(np.int16)
        ldst[pos] = l_s.astype(np.float32)
        val[pos] = v_s.astype(np.float32)

        # wrap indices: idx i -> [i%16, i//16], replicated on all 8 q7 cores
        idxw = np.tile(idx.reshape(-1, 16).T, (8, 1)).copy()          # [128, L/16]
        ldstT = np.ascontiguousarray(ldst.reshape(-1, 128).T)        # [128, CHUNKS]
        valT = np.ascontiguousarray(val.reshape(-1, 128).T)
        idx_all.append(idxw)
        ldst_all.append(ldstT)
        val_all.append(valT)
    return idx_all, ldst_all, val_all


def _pack_x(cfg, x, slot):
    """x [N, IN] -> per-core padded transposed shards [IN, PADSHARD],
    rows placed at their balanced slots."""
    xp = np.zeros((cfg.NPAD, cfg.IN_DIM), dtype=np.float32)
    xp[slot] = x
    shards = []
    for c in range(cfg.M):
        xT = np.ascontiguousarray(
            xp[c * cfg.PADSHARD:(c + 1) * cfg.PADSHARD].T)
        if cfg.X_BF16:
            xT = xT.astype(ml_dtypes.bfloat16)
        shards.append(xT)
    return shards


# --------------------------------------------------------- device program ---
def _build(cfg, timing=False):
    from concourse import bacc, tile
    import concourse.mybir as mybir

    f32 = mybir.dt.float32
    bf16 = mybir.dt.bfloat16
    i16 = mybir.dt.int16
    AOP = mybir.AluOpType
    ACT = mybir.ActivationFunctionType

    xdt = bf16 if cfg.X_BF16 else f32

    nc = bacc.Bacc("TRN2", target_bir_lowering=False, debug=False,
                   num_devices=1 if timing else cfg.M,
                   dynamic_dma_scratch_size=getattr(cfg, "DMA_SCRATCH", 16384),
                   num_swdge_queues=getattr(cfg, "NQUEUES", 1))

    NPAD, QROWS, NT, SLABC = cfg.NPAD, cfg.QROWS, cfg.NT, cfg.SLABC
    KSEGT, CQQ, QCOFF, NSLABQ = cfg.KSEGT, cfg.CQQ, cfg.QCOFF, cfg.NSLABQ
    CUMT = cfg.CUMT
    CHUNKS, HID, NCLS, IN_DIM = cfg.CHUNKS, cfg.HID, cfg.NCLS, cfg.IN_DIM
    LQ16Q = [cq * 128 // 16 for cq in CQQ]   # idx columns per quarter
    QIOFF = [o * 128 // 16 for o in QCOFF]   # idx column offset per quarter
    SLAB16 = SLABC * 128 // 16               # idx columns per slab
    NA = cfg.ASLAB                     # phase-A slabs (1792 nodes each)

    # -------- I/O
    XT = nc.dram_tensor("xt", [IN_DIM, cfg.PADSHARD], xdt, kind="ExternalInput")
    IDX = nc.dram_tensor("idx", [128, CHUNKS * 8], i16, kind="ExternalInput")
    LDST = nc.dram_tensor("ldst", [128, CHUNKS], f32, kind="ExternalInput")
    VAL = nc.dram_tensor("val", [128, CHUNKS], f32, kind="ExternalInput")
    W1 = nc.dram_tensor("w1", [IN_DIM, HID], xdt, kind="ExternalInput")
    W2 = nc.dram_tensor("w2", [HID, HID], f32, kind="ExternalInput")
    WC = nc.dram_tensor("wc", [HID, NCLS], f32, kind="ExternalInput")
    B1 = nc.dram_tensor("b1", [128, HID], f32, kind="ExternalInput")   # replicated
    B2 = nc.dram_tensor("b2", [128, HID], f32, kind="ExternalInput")
    BC = nc.dram_tensor("bc", [128, NCLS], f32, kind="ExternalInput")
    IOTA = nc.dram_tensor("iota", [128, 128], bf16, kind="ExternalInput")
    IDENT = nc.dram_tensor("ident", [128, 128], f32, kind="ExternalInput")
    # transposed layout: OUT[p, t*NCLS+c] = node (t*128+p) class c
    OUT = nc.dram_tensor("out", [128, NT * NCLS], f32, kind="ExternalOutput")

    # -------- internal DRAM
    T1S = nc.dram_tensor("t1shard", [cfg.PADSHARD, 128], bf16)      # cols 64: junk
    T1F = nc.dram_tensor("t1full", [NPAD, 128], bf16, addr_space="Shared")
    T2S = nc.dram_tensor("t2shard", [cfg.PADSHARD, 128], bf16)
    T2F = nc.dram_tensor("t2full", [NPAD, 128], bf16, addr_space="Shared")

    with tile.TileContext(nc) as tc, ExitStack() as top:
        # gather-critical stream data first: idx unlocks the first gathers
        edg = top.enter_context(tc.tile_pool(name="edg", bufs=1))
        its = []
        for q in range(4):
            it = edg.tile([128, LQ16Q[q]], i16, tag=f"idx{q}")
            nc.sync.dma_start(out=it, in_=IDX[:, QIOFF[q]:QIOFF[q] + LQ16Q[q]])
            its.append(it)
        ldsts = edg.tile([128, CHUNKS], f32)
        nc.sync.dma_start(out=ldsts, in_=LDST[:, :])
        vals = edg.tile([128, CHUNKS], f32)
        nc.sync.dma_start(out=vals, in_=VAL[:, :])

        cpool = top.enter_context(tc.tile_pool(name="consts", bufs=1))
        iot = cpool.tile([128, 128], bf16)
        nc.sync.dma_start(out=iot, in_=IOTA[:, :])
        w1s = cpool.tile([IN_DIM, HID], xdt)
        nc.sync.dma_start(out=w1s, in_=W1[:, :])
        w2s = cpool.tile([HID, HID], f32)
        nc.sync.dma_start(out=w2s, in_=W2[:, :])
        wcs = cpool.tile([HID, NCLS], f32)
        nc.sync.dma_start(out=wcs, in_=WC[:, :])
        b1s = cpool.tile([128, HID], f32)
        nc.sync.dma_start(out=b1s, in_=B1[:, :])
        b2s = cpool.tile([128, HID], f32)
        nc.sync.dma_start(out=b2s, in_=B2[:, :])
        bcs = cpool.tile([128, NCLS], f32)
        nc.sync.dma_start(out=bcs, in_=BC[:, :])
        b17s = cpool.tile([128, 7, HID], f32)
        for r in range(7):
            nc.sync.dma_start(out=b17s[:, r, :], in_=B1[:, :])
        idn = cpool.tile([128, 128], f32)
        nc.sync.dma_start(out=idn, in_=IDENT[:, :])

        accp = top.enter_context(tc.tile_pool(name="acc", bufs=1))

        # ====== phase A: T1S = x_shard @ W1 + b1 (node-major bf16 rows),
        # then AllGather into the full table T1F.
        with tc.tile_pool(name="xa", bufs=3) as xa, \
             tc.tile_pool(name="sta", bufs=3) as sta, \
             tc.tile_pool(name="psa", bufs=4, space="PSUM") as psa:
            for s in range(NA):
                xs = xa.tile([128, 1792], xdt)
                nc.sync.dma_start(out=xs, in_=XT[:, s * 1792:(s + 1) * 1792])
                st = sta.tile([128, 14, HID], bf16)
                for h in range(2):
                    pb = psa.tile([128, 7, HID], f32)
                    for k7 in range(7):
                        k = h * 7 + k7
                        nc.tensor.matmul(pb[:, k7, :],
                                         lhsT=xs[:, k * 128:(k + 1) * 128],
                                         rhs=w1s, start=True, stop=True)
                    nc.vector.tensor_tensor(st[:, h * 7:(h + 1) * 7, :], pb,
                                            b17s, AOP.add)
                dst = T1S[s * 1792:(s + 1) * 1792, 0:HID].rearrange(
                    "(k p) f -> p k f", p=128)
                nc.sync.dma_start(out=dst, in_=st)
        if not timing:
            nc.gpsimd.collective_compute(
                "AllGather", mybir.AluOpType.bypass,
                replica_groups=[list(range(cfg.M))],
                ins=[T1S[:, :]], outs=[T1F[:, :]])

        # ============ spmm layer runner: per-tile single psum group across
        # all 4 quarters (slabs for all quarters retire in lockstep), with a
        # fused per-tile epilogue.
        def spmm_layer(tab, epilogue, pools):
            msg, vp, psb = pools
            slabs = [[None] * NSLABQ[q] for q in range(4)]
            nslab_tot = sum(NSLABQ)

            def ensure_slab(q, s):
                if s >= NSLABQ[q]:
                    return None
                if slabs[q][s] is None:
                    sc = min(SLABC, CQQ[q] - s * SLABC)   # last slab: partial
                    mt = msg.tile([128, SLABC, 128], bf16)
                    nc.gpsimd.dma_gather(
                        mt[:, 0:sc, :], tab[q * QROWS:(q + 1) * QROWS, :],
                        its[q][:, s * SLAB16:s * SLAB16 + sc * 8],
                        num_idxs=sc * 128, num_idxs_reg=sc * 128,
                        elem_size=128, elem_step=128,
                        single_packet=getattr(cfg, "SINGLE_PACKET", True),
                        queue_num=(q * NSLABQ[0] + s) % getattr(cfg, "NQUEUES", 1))
                    slabs[q][s] = mt
                return slabs[q][s]

            # prefetch: keep PREGEN slab generations in flight ahead of use
            PREGEN = getattr(cfg, "PREGEN", 2)
            for g in range(PREGEN):
                for q in range(4):
                    ensure_slab(q, g)

            for t in range(NT):
                for q in range(4):
                    ensure_slab(q, CUMT[q][t] // SLABC + PREGEN)
                ps = psb.tile([128, HID], f32)
                for q in range(4):
                    for k in range(KSEGT[t][q]):
                        j = CUMT[q][t] + k               # chunk in quarter
                        gj = QCOFF[q] + j                # global chunk
                        v = vp.tile([128, 128], bf16)
                        nc.vector.tensor_scalar(
                            v, iot, ldsts[:, gj:gj + 1], vals[:, gj:gj + 1],
                            AOP.is_equal, AOP.mult)
                        mt = ensure_slab(q, j // SLABC)
                        nc.tensor.matmul(ps, lhsT=v,
                                         rhs=mt[:, j % SLABC, 0:HID],
                                         start=(q == 0 and k == 0),
                                         stop=(q == 3 and
                                               k == KSEGT[t][3] - 1))
                epilogue(t, ps)

        # ================= layer 1 + phase C fused: T2S = relu(h1)@W2+b2
        for _rep in range(getattr(cfg, "REPS", 1)):
            _run_layers(cfg, nc, tc, tile, mybir, timing, accp, locals())
    nc.compile()
    return nc


def _run_layers(cfg, nc, tc, tile, mybir, timing, accp, env):
    f32 = mybir.dt.float32
    bf16 = mybir.dt.bfloat16
    i16 = mybir.dt.int16
    AOP = mybir.AluOpType
    ACT = mybir.ActivationFunctionType
    NPAD, QROWS, NT, SLABC = cfg.NPAD, cfg.QROWS, cfg.NT, cfg.SLABC
    CHUNKS, HID, NCLS, IN_DIM = cfg.CHUNKS, cfg.HID, cfg.NCLS, cfg.IN_DIM
    (T1F, T2S, T2F, IDX, OUT, iot, idn, ldsts, vals, w2s, wcs, b2s, bcs) = (
        env["T1F"], env["T2S"], env["T2F"], env["IDX"], env["OUT"], env["iot"],
        env["idn"], env["ldsts"], env["vals"], env["w2s"], env["wcs"],
        env["b2s"], env["bcs"])
    spmm_layer = env["spmm_layer"]

    if True:
        with tc.tile_pool(name="msg", bufs=getattr(cfg, "MSGBUFS", 8)) as msg, \
             tc.tile_pool(name="vp", bufs=8) as vp, \
             tc.tile_pool(name="psb", bufs=3, space="PSUM") as psb, \
             tc.tile_pool(name="tc1", bufs=3) as tp1, \
             tc.tile_pool(name="tc2", bufs=3) as tp2, \
             tc.tile_pool(name="tc3", bufs=3) as tp3, \
             tc.tile_pool(name="pst", bufs=2, space="PSUM") as pst, \
             tc.tile_pool(name="psc", bufs=2, space="PSUM") as psc:

            def epi1(t, ps):
                h1r = tp1.tile([128, HID], f32)
                nc.scalar.activation(h1r, ps, ACT.Relu)
                ptr = pst.tile([HID, 128], f32)
                nc.tensor.transpose(ptr, h1r, idn)
                h1t = tp2.tile([HID, 128], f32)
                nc.vector.tensor_copy(out=h1t, in_=ptr)
                ps2 = psc.tile([128, HID], f32)
                nc.tensor.matmul(ps2, lhsT=h1t, rhs=w2s, start=True, stop=True)
                t2t = tp3.tile([128, HID], bf16)
                nc.vector.tensor_tensor(t2t, ps2, b2s, AOP.add)
                nc.sync.dma_start(out=T2S[t * 128:(t + 1) * 128, 0:HID], in_=t2t)

            spmm_layer(T1F, epi1, (msg, vp, psb))
            if not timing:
                nc.gpsimd.collective_compute(
                    "AllGather", mybir.AluOpType.bypass,
                    replica_groups=[list(range(cfg.M))],
                    ins=[T2S[:, :]], outs=[T2F[:, :]])

        # ================= layer 2 + phase E fused: logits + log_softmax
        with tc.tile_pool(name="msg2", bufs=getattr(cfg, "MSGBUFS", 8)) as msg2, \
             tc.tile_pool(name="vp2", bufs=8) as vp2, \
             tc.tile_pool(name="psb2", bufs=3, space="PSUM") as psb2, \
             tc.tile_pool(name="te1", bufs=3) as te1, \
             tc.tile_pool(name="te2", bufs=3) as te2, \
             tc.tile_pool(name="pse", bufs=2, space="PSUM") as pse, \
             tc.tile_pool(name="psf", bufs=2, space="PSUM") as psf:
            lgacc = accp.tile([128, NT, NCLS], f32, tag="lgacc")
            negmacc = accp.tile([128, NT], f32, tag="negmacc")
            smacc = accp.tile([128, NT], f32, tag="smacc")

            lnacc = accp.tile([128, NT], f32, tag="lnacc")
            shacc = accp.tile([128, NT], f32, tag="shacc")
            FBLK = 14                      # tiles per finalize block

            def epi2(t, ps):
                h2s = te1.tile([128, HID], f32)
                nc.scalar.activation(h2s, ps, ACT.Copy)
                ptr = pse.tile([HID, 128], f32)
                nc.tensor.transpose(ptr, h2s, idn)
                h2t = te2.tile([HID, 128], f32)
                nc.vector.tensor_copy(out=h2t, in_=ptr)
                psl = psf.tile([128, NCLS], f32)
                nc.tensor.matmul(psl, lhsT=h2t, rhs=wcs, start=True, stop=True)
                nc.vector.tensor_tensor(lgacc[:, t, :], psl, bcs, AOP.add)
                nc.vector.tensor_reduce(negmacc[:, t:t + 1], lgacc[:, t, :],
                                        mybir.AxisListType.X, AOP.max,
                                        negate=True)
                et = te1.tile([128, NCLS], f32, tag="et")
                nc.scalar.activation(et, lgacc[:, t, :], ACT.Exp,
                                     bias=negmacc[:, t:t + 1],
                                     accum_out=smacc[:, t:t + 1])
                if (t + 1) % FBLK == 0:
                    # finalize block: log_softmax shift + store
                    b = t + 1 - FBLK
                    nc.scalar.activation(lnacc[:, b:t + 1], smacc[:, b:t + 1],
                                         ACT.Ln)
                    nc.vector.tensor_tensor(shacc[:, b:t + 1],
                                            lnacc[:, b:t + 1],
                                            negmacc[:, b:t + 1], AOP.subtract)
                    for u in range(b, t + 1):
                        nc.vector.tensor_scalar(lgacc[:, u, :], lgacc[:, u, :],
                                                shacc[:, u:u + 1], None,
                                                AOP.subtract)
                    nc.sync.dma_start(
                        out=OUT[:, b * NCLS:(t + 1) * NCLS],
                        in_=lgacc[:, b:t + 1, :])

            spmm_layer(T2F, epi2, (msg2, vp2, psb2))

    nc.compile()
    return nc


_NC_CACHE = {}
_PLAN_CACHE = {}


def _get_nc(cfg):
    key = (cfg.KSEGT, cfg.X_BF16, getattr(cfg, "REPS", 1), cfg.SLABC,
           cfg.MSGBUFS, cfg.PREGEN)
    if key not in _NC_CACHE:
        _NC_CACHE[key] = _build(cfg)
    return _NC_CACHE[key]


# ------------------------------------------------------------------ main ---
def kernel(x, edge_row, edge_col, edge_val, W1, b1, W2, b2, Wc, bc,
           _run_kwargs=None):
    from concourse.bass_utils import run_bass_kernel_spmd

    cfg = CFG
    x = np.asarray(x, dtype=np.float32)
    edge_row = np.asarray(edge_row, dtype=np.int64)
    edge_col = np.asarray(edge_col, dtype=np.int64)
    edge_val = np.asarray(edge_val, dtype=np.float32)
    W1 = np.asarray(W1, dtype=np.float32)
    W2 = np.asarray(W2, dtype=np.float32)
    Wc = np.asarray(Wc, dtype=np.float32)
    b1 = np.asarray(b1, dtype=np.float32)
    b2 = np.asarray(b2, dtype=np.float32)
    bc = np.asarray(bc, dtype=np.float32)

    import hashlib
    h = hashlib.md5()
    h.update(np.ascontiguousarray(edge_row).tobytes())
    h.update(np.ascontiguousarray(edge_col).tobytes())
    fp = h.hexdigest()
    if fp in _PLAN_CACHE:
        slot, ksegt = _PLAN_CACHE[fp]
    else:
        slot, ksegt = _balance(cfg, edge_row, edge_col)
        _PLAN_CACHE[fp] = (slot, ksegt)
    cfg.KSEGT = ksegt

    idx_all, ldst_all, val_all = _plan(cfg, edge_row, edge_col, edge_val, slot)
    xT = _pack_x(cfg, x, slot)
    w1h = W1.astype(ml_dtypes.bfloat16) if cfg.X_BF16 else W1
    iota = np.tile(np.arange(128, dtype=np.float32), (128, 1)).astype(
        ml_dtypes.bfloat16)
    ident = np.eye(128, dtype=np.float32)
    b1r = np.tile(b1, (128, 1)).astype(np.float32)
    b2r = np.tile(b2, (128, 1)).astype(np.float32)
    bcr = np.tile(bc, (128, 1)).astype(np.float32)

    nc = _get_nc(cfg)
    in_maps = []
    for c in range(cfg.M):
        in_maps.append({
            "xt": xT[c], "idx": idx_all[c], "ldst": ldst_all[c],
            "val": val_all[c], "w1": w1h, "w2": W2, "wc": Wc,
            "b1": b1r, "b2": b2r, "bc": bcr, "iota": iota, "ident": ident,
        })
    kw = dict(_run_kwargs or {})
    res = run_bass_kernel_spmd(nc, in_maps, core_ids=list(range(cfg.M)), **kw)
    rows = []
    for c in range(cfg.M):
        o = np.asarray(res.results[c]["out"])          # [128, NT*NCLS]
        rows.append(o.reshape(128, cfg.NT, cfg.NCLS).transpose(1, 0, 2)
                    .reshape(cfg.PADSHARD, cfg.NCLS))
    allrows = np.concatenate(rows, axis=0)             # [NPAD, NCLS]
    out = allrows[slot]                                # unpermute to node order
    kernel.last_results = res
    return out.astype(np.float32)

